# revision 15
# baseline (speedup 1.0000x reference)
"""HAN (heterogeneous graph attention) kernel for nn_BNNHAN_11038065951338.

Runs on 8 Trainium2 NeuronCores via Bass (compiled once at import, executed
through a persistent jax/PJRT executable). kernel(**inputs) takes the FULL
unsharded numpy inputs and returns the FULL [100000, 2] float32 output.

Per core (dst-partitioned, 1/8 of subject nodes):
  P0: AllGather bf16 x tables; build a_dst table; memset intermediates.
  P1: edge phase — per (slotbank, srcbank, parity) group of edges:
      pair-transpose-gather x_src, gather a_dst, project+attend on PE,
      exp(lrelu) on ACT, scatter msg||exp to a unique dst-sorted slot.
  P2: accumulation — read slots in dst order, dedup each 128-slot tile
      with a self-match matrix matmul, scatter one row per dst segment.
  P3: normalize (softmax denom), relu, semantic-attention partials and
      2-dim output projections.
Host: bucket/sort edges, assemble int16 index arrays; final 2-way
metapath softmax combine.
"""

import dataclasses
import time

import numpy as np
import ml_dtypes

import jax as _jax
try:
    _jax.config.update("jax_compilation_cache_dir", "/tmp/jax_cache_han")
    _jax.config.update("jax_persistent_cache_min_entry_size_bytes", -1)
    _jax.config.update("jax_persistent_cache_min_compile_time_secs", 0.0)
except Exception:
    pass

import concourse.bass as bass
import concourse.bacc as bacc
import concourse.mybir as mybir
import concourse.tile as tile
import bass_rust
from concourse.masks import make_identity

bf16 = np.float16
F32 = mybir.dt.float32
BF = mybir.dt.float16
I16 = mybir.dt.int16
AXX = bass_rust.AxisListType.X
N_CORES = 8
H, DH, F = 8, 16, 128
DIN = 64
MP = ("cs", "ss")


@dataclasses.dataclass
class P:
    NNODE: int = 100000          # nodes per type
    PART: int = 12500            # dst nodes per core
    PAIR_BANK: int = 16384       # gather idx per src bank (pairs)
    W: int = 16                  # dst nodes per 128-slot window
    WPB: int = 224               # windows per slot bank
    SLOT_BANK: int = 32768       # slot-idx space per scatter bank
    ACC_CHUNK: int = 4096        # slots per accumulation chunk
    CAPS_TBL: tuple = ((3328, 3328, 3328, 256), (3328, 3328, 3328, 256),
                       (3328, 3328, 3328, 256), (1792, 1792, 1792, 256))

    @property
    def N_SRC_BANK(self):
        return max(1, -(-(self.NNODE // 2) // self.PAIR_BANK))

    @property
    def N_WIN(self):
        return -(-self.PART // self.W)

    @property
    def N_SLOT_BANK(self):
        return -(-self.N_WIN // self.WPB)

    @property
    def SLOT_REAL(self):
        return self.WPB * 128

    def wb(self, b):
        return min(self.WPB, self.N_WIN - self.WPB * b)

    def chunks_b(self, b):
        return -(-self.wb(b) * 128 // self.ACC_CHUNK)

    @property
    def ACC_REGIONS(self):
        return [(b, ch) for b in range(self.N_SLOT_BANK)
                for ch in range(self.chunks_b(b))]

    @property
    def ASLOTS(self):
        return len(self.ACC_REGIONS) * self.ACC_CHUNK

    @property
    def N_GROUP(self):
        return self.N_SLOT_BANK * self.N_SRC_BANK * 2

    def capg(self, g):
        return self.CAPS_TBL[g // (self.N_SRC_BANK * 2)][
            (g // 2) % self.N_SRC_BANK]

    @property
    def CUMCAPS(self):
        import itertools
        return [0] + list(itertools.accumulate(
            self.capg(g) for g in range(self.N_GROUP)))

    @property
    def ECAP(self):
        return self.CUMCAPS[-1]

    @property
    def ACC_SUB(self):
        return self.ACC_CHUNK // 128

    @property
    def DST_TILES(self):
        return (self.PART + 127) // 128

    @property
    def LAST_TILE_N(self):
        return self.PART - (self.DST_TILES - 1) * 128

    @property
    def FT_ROWS(self):
        return self.DST_TILES * 128 + 128

    @property
    def DUMP_DST(self):
        return self.DST_TILES * 128 + 9

    @property
    def AD_PAD(self):
        return self.PART + 8


FULL = P()
SMALL = P(NNODE=2048, PART=256, PAIR_BANK=1024, W=16, WPB=6,
          SLOT_BANK=1024, ACC_CHUNK=1024,
          CAPS_TBL=((384,), (384,), (256,)))


def build(p: P, phases: int = 3, p1mode: int = 0):
    nc = bacc.Bacc("TRN2", target_bir_lowering=False, debug=False,
                   num_devices=N_CORES)
    NPAIR = p.NNODE // 2

    xs_sh = nc.dram_tensor("xs_sh", [p.PART, DIN], BF, kind="ExternalInput")
    xc_sh = nc.dram_tensor("xc_sh", [p.PART, DIN], BF, kind="ExternalInput")
    E16 = p.ECAP // 16
    ins = {}
    for m in MP:
        ins[f"eidx_{m}"] = nc.dram_tensor(f"eidx_{m}", [16, 3 * E16],
                                          I16, kind="ExternalInput")
        ins[f"scatidx_{m}"] = nc.dram_tensor(
            f"scatidx_{m}", [16, p.ASLOTS // 16], I16, kind="ExternalInput")
        ins[f"dstrow_{m}"] = nc.dram_tensor(
            f"dstrow_{m}", [p.ASLOTS // p.ACC_CHUNK, p.ACC_CHUNK], I16,
            kind="ExternalInput")
    cblob = nc.dram_tensor("cblob", [128, 691], BF, kind="ExternalInput")

    pout = nc.dram_tensor("pout", [2, 2 * p.DST_TILES * 128 + 128],
                          F32, kind="ExternalOutput")
    p_out = {m: pout[:, mi * p.DST_TILES * 128:
                     (mi + 1) * p.DST_TILES * 128]
             for mi, m in enumerate(MP)}

    xs_b = nc.dram_tensor("xs_b", [p.PART, DIN], BF)
    xc_b = nc.dram_tensor("xc_b", [p.PART, DIN], BF)
    xs_full = nc.dram_tensor("xs_full", [p.NNODE, DIN], BF,
                             addr_space="Shared")
    xc_full = nc.dram_tensor("xc_full", [p.NNODE, DIN], BF,
                             addr_space="Shared")
    ad_tab = nc.dram_tensor("ad_tab", [p.PART + 64, DIN], F32)
    inter = {m: nc.dram_tensor(f"inter_{m}", [p.N_SLOT_BANK * p.SLOT_BANK, 256], BF) for m in MP}
    ftab = {m: nc.dram_tensor(f"ftab_{m}", [p.FT_ROWS, 256], BF) for m in MP}
    rep = {}
    for m in MP:
        for nm, ncol in (("eidx", 3 * E16), ("scatidx", p.ASLOTS // 16)):
            rep[f"{nm}_{m}"] = nc.dram_tensor(f"r{nm}_{m}", [128, ncol], I16)

    x_full = {"cs": xc_full, "ss": xs_full}

    with tile.TileContext(nc) as tc:
        with (
            tc.tile_pool(name="consts", bufs=1) as cpool,
            tc.tile_pool(name="sbuf", bufs=2) as pool,
            tc.tile_pool(name="acc1", bufs=1) as apool,
            tc.tile_pool(name="psum", bufs=8, space="PSUM") as pp,
        ):
            # ================= P0 =================
            nc.sync.dma_start(out=xs_b[:], in_=xs_sh[:])
            nc.sync.dma_start(out=xc_b[:], in_=xc_sh[:])
            nc.gpsimd.collective_compute(
                "AllGather", mybir.AluOpType.bypass,
                replica_groups=[list(range(N_CORES))],
                ins=[xs_b[:]], outs=[xs_full[:]])
            nc.gpsimd.collective_compute(
                "AllGather", mybir.AluOpType.bypass,
                replica_groups=[list(range(N_CORES))],
                ins=[xc_b[:]], outs=[xc_full[:]])

            for m in MP:
                for nm in ("eidx", "scatidx"):
                    for k in range(8):
                        nc.sync.dma_start(
                            out=rep[f"{nm}_{m}"][16 * k:16 * (k + 1), :],
                            in_=ins[f"{nm}_{m}"][:])

            ident = cpool.tile([128, 128], BF)
            make_identity(nc, ident[:])
            identf = cpool.tile([128, 128], F32)
            make_identity(nc, identf[:])
            cb = cpool.tile([128, 691], BF)
            nc.sync.dma_start(out=cb[:], in_=cblob[:])
            wcat_t = {}
            for mi_, m in enumerate(MP):
                for par in range(2):
                    off = (mi_ * 2 + par) * 136
                    wcat_t[(m, par)] = cb[:, off:off + 136]
            wad_t = cb[0:DIN, 544:560]
            kw_t = cb[:, 560:688]
            linw_t = cb[:, 688:690]
            kb_t = cb[:, 690:691]

            # a_d table
            for t_i in range(p.DST_TILES):
                lo = t_i * 128
                n = min(128, p.PART - lo)
                xst = pool.tile([128, DIN], BF, tag="xst")
                if n < 128:
                    nc.vector.memset(xst[:], 0.0)
                nc.sync.dma_start(out=xst[:n, :], in_=xs_sh[lo:lo + n, :])
                tp = pp.tile([DIN, 128], BF, tag="ps")
                nc.tensor.transpose(out=tp[:], in_=xst[:], identity=ident[:])
                xstT = pool.tile([DIN, 128], BF, tag="xstT")
                nc.vector.tensor_copy(out=xstT[:], in_=tp[:])
                adp = pp.tile([128, 16], F32, tag="ps")
                nc.tensor.matmul(out=adp[:], lhsT=xstT[:], rhs=wad_t,
                                 start=True, stop=True)
                ads = pool.tile([128, 16], F32, tag="ads")
                nc.vector.tensor_copy(out=ads[:], in_=adp[:])
                nc.sync.dma_start(out=ad_tab[lo:lo + n, 0:16], in_=ads[:n, :])
            padt = cpool.tile([128, DIN], F32)
            nc.vector.memset(padt[:], -100.0)
            nc.sync.dma_start(out=ad_tab[p.PART:p.PART + 64, :],
                              in_=padt[0:64, :])

            zt = cpool.tile([128, 16, 256], BF)
            nc.vector.memset(zt[:], 0.0)
            for m in MP:
                iv = inter[m][:].rearrange("(a q) c -> q a c", q=128)
                for b in range(p.N_SLOT_BANK):
                    base = b * p.SLOT_BANK // 128
                    nreg = p.chunks_b(b) * p.ACC_CHUNK // 128
                    ndmp = (p.SLOT_BANK - p.SLOT_REAL) // 128
                    for k0, k1 in ((base, base + nreg),
                                   (base + p.SLOT_REAL // 128,
                                    base + p.SLOT_REAL // 128 + ndmp)):
                        for k in range(k0, k1, 16):
                            w = min(16, k1 - k)
                            nc.sync.dma_start(out=iv[:, k:k + w, :],
                                              in_=zt[:, :w, :])
                fv = ftab[m][:].rearrange("(a q) c -> q a c", q=128)
                nfa = p.FT_ROWS // 128
                for k in range(0, nfa, 16):
                    w = min(16, nfa - k)
                    nc.sync.dma_start(out=fv[:, k:k + w, :],
                                      in_=zt[:, :w, :])

            # ================= P1: edge phase =================
            for m in (MP if phases >= 1 else ()):
                NSB = p.N_SRC_BANK
                for g in range(p.N_GROUP):
                    sb_bank = g // (2 * NSB)
                    srcb = (g // 2) % NSB
                    par = g % 2
                    CAP = p.capg(g)
                    EPS = CAP // 128
                    c0 = p.CUMCAPS[g] // 16
                    ncol = CAP // 16
                    E16_ = p.ECAP // 16
                    gi = pool.tile([128, ncol], I16, tag="gi")
                    nc.sync.dma_start(
                        out=gi[:], in_=rep[f"eidx_{m}"][:, c0:c0 + ncol])
                    di = pool.tile([128, ncol], I16, tag="di")
                    nc.sync.dma_start(
                        out=di[:],
                        in_=rep[f"eidx_{m}"][:, E16_ + c0:E16_ + c0 + ncol])
                    si = pool.tile([128, ncol], I16, tag="si")
                    nc.sync.dma_start(
                        out=si[:],
                        in_=rep[f"eidx_{m}"][:, 2 * E16_ + c0:
                                             2 * E16_ + c0 + ncol])

                    xg = pool.tile([128, EPS, 128], BF, tag="xg")
                    bank_ap = bass.AP(
                        x_full[m][:].tensor, srcb * p.PAIR_BANK * 128,
                        [[128, NPAIR - srcb * p.PAIR_BANK], [1, 128]])
                    if p1mode < 2:
                        nc.gpsimd.dma_gather(
                            xg[:], bank_ap, gi[:], CAP, CAP, 128,
                            single_packet=False)
                    else:
                        nc.vector.memset(xg[:], 0.25)
                    ad = pool.tile([128, EPS, DIN], F32, tag="ad")
                    if p1mode < 1:
                        nc.gpsimd.dma_gather(
                            ad[:], ad_tab[:], di[:], CAP, CAP, DIN,
                            single_packet=False)
                    else:
                        nc.vector.memset(ad[:], 0.25)

                    talpha = pool.tile([128, EPS * 8], F32, tag="ta")
                    etile = pool.tile([128, EPS * 8], F32, tag="et")
                    scb = pool.tile([128, EPS, 256], BF, tag="scb")
                    adoff = 0 if m == "cs" else 8
                    GB = 4  # psum group batch
                    for s0 in range(0, EPS, GB):
                        s1 = min(s0 + GB, EPS)
                        hps = []
                        for s in range(s0, s1):
                            xtp = pp.tile([128, 128], BF, tag="ps")
                            nc.tensor.transpose(out=xtp[:], in_=xg[:, s, :],
                                                identity=ident[:])
                            xts = pool.tile([128, 128], BF, tag="xts")
                            nc.vector.tensor_copy(out=xts[:], in_=xtp[:])
                            hp = pp.tile([128, 136], F32, tag="ps")
                            nc.tensor.matmul(
                                out=hp[:], lhsT=xts[:],
                                rhs=wcat_t[(m, par)],
                                start=True, stop=True)
                            nc.vector.tensor_tensor(
                                out=talpha[:, 8 * s:8 * (s + 1)],
                                in0=hp[:, 128:136],
                                in1=ad[:, s, adoff:adoff + 8],
                                op=mybir.AluOpType.add)
                            hps.append(hp)
                        nsub = s1 - s0
                        tsl = talpha[:, 8 * s0:8 * s1]
                        usl = etile[:, 8 * s0:8 * s1]
                        nc.vector.tensor_scalar_mul(usl, tsl, 0.2)
                        nc.vector.tensor_tensor(out=tsl, in0=tsl, in1=usl,
                                                op=mybir.AluOpType.max)
                        nc.scalar.activation(
                            out=usl, in_=tsl,
                            func=mybir.ActivationFunctionType.Exp)
                        esl = scb[:, s0, 128:136]
                        eout = bass.AP(esl.tensor, esl.offset,
                                       [esl.ap[0], [256, nsub], [1, 8]])
                        nc.vector.tensor_copy(out=eout,
                                              in_=etile[:, 8 * s0:8 * s1])
                        for s in range(s0, s1):
                            ea = etile[:, 8 * s:8 * s + 8]
                            ebc = bass.AP(ea.tensor, ea.offset,
                                          [ea.ap[0], [1, 8], [0, 16]])
                            nc.vector.tensor_tensor(
                                out=scb[:, s, 0:128],
                                in0=hps[s - s0][:, 0:128], in1=ebc,
                                op=mybir.AluOpType.mult)
                    if p1mode < 3:
                        nc.gpsimd.dma_scatter_add(
                            inter[m][sb_bank * p.SLOT_BANK:
                                     (sb_bank + 1) * p.SLOT_BANK, :],
                            scb[:], si[:], CAP, CAP, 256,
                            single_packet=False)

            # ================= P2: accumulation =================
            for m in (MP if phases >= 2 else ()):
                dcol_i = apool.tile([128, p.ASLOTS // 128], I16,
                                    tag=f"dcoli{m}")
                nc.sync.dma_start(
                    out=dcol_i[:],
                    in_=ins[f"dstrow_{m}"][:].rearrange(
                        "a (b c) -> (a b) c", c=128),
                    transpose=True)
                for ch, (rb, rch) in enumerate(p.ACC_REGIONS):
                    a0 = (rb * p.SLOT_BANK + rch * p.ACC_CHUNK) // 128
                    me = pool.tile([128, p.ACC_SUB, 256], BF, tag="me")
                    nc.sync.dma_start(
                        out=me[:],
                        in_=inter[m][:].rearrange("(a q) c -> q a c", q=128)[
                            :, a0:a0 + p.ACC_SUB, :])
                    sci = pool.tile([128, p.ACC_CHUNK // 16], I16, tag="sci")
                    nc.sync.dma_start(
                        out=sci[:],
                        in_=rep[f"scatidx_{m}"][
                            :, ch * (p.ACC_CHUNK // 16):
                            (ch + 1) * (p.ACC_CHUNK // 16)])
                    rt = pool.tile([128, p.ACC_CHUNK], I16, tag="rt")
                    rsrc = bass.AP(ins[f"dstrow_{m}"][:].tensor,
                                   ch * p.ACC_CHUNK,
                                   [[0, 128], [1, p.ACC_CHUNK]])
                    nc.sync.dma_start(out=rt[:], in_=rsrc)
                    scat = pool.tile([128, p.ACC_SUB, 256], BF, tag="scat")
                    for s in range(p.ACC_SUB):
                        st = ch * p.ACC_SUB + s
                        csl = dcol_i[:, st:st + 1]
                        col_bc = bass.AP(csl.tensor, csl.offset,
                                         [csl.ap[0], [0, 128]])
                        S = pool.tile([128, 128], BF, tag="S")
                        nc.vector.tensor_tensor(
                            out=S[:], in0=col_bc,
                            in1=rt[:, 128 * s:128 * (s + 1)],
                            op=mybir.AluOpType.is_equal)
                        dp = pp.tile([128, 136], F32, tag="ps")
                        nc.tensor.matmul(out=dp[:], lhsT=S[:],
                                         rhs=me[:, s, 0:136],
                                         start=True, stop=True)
                        nc.vector.tensor_copy(out=scat[:, s, 0:136],
                                              in_=dp[:])
                    nc.gpsimd.dma_scatter_add(
                        ftab[m][:], scat[:], sci[:],
                        p.ACC_CHUNK, p.ACC_CHUNK, 256,
                        single_packet=False)

            # ================= P3: epilogue =================
            tsum_t = apool.tile([128, 2], F32, tag="tsum")
            nc.vector.memset(tsum_t[:], 0.0)
            for mi, m in enumerate(MP if phases >= 3 else ()):
                for t_i in range(p.DST_TILES):
                    ft = pool.tile([128, 136], BF, tag="ft")
                    nc.sync.dma_start(
                        out=ft[:],
                        in_=ftab[m][:].rearrange("(a q) c -> q a c", q=128)[
                            :, t_i, 0:136])
                    sden = pool.tile([128, 8], F32, tag="sden")
                    nc.vector.tensor_scalar_add(sden[:], ft[:, 128:136],
                                                1e-16)
                    rec = pool.tile([128, 8], F32, tag="rec")
                    nc.vector.reciprocal(rec[:], sden[:])
                    o32 = pool.tile([128, 128], F32, tag="o32")
                    rbc = bass.AP(rec.tensor, rec[:].offset,
                                  [rec[:].ap[0], [1, 8], [0, 16]])
                    nc.vector.tensor_tensor(out=o32[:], in0=ft[:, 0:128],
                                            in1=rbc,
                                            op=mybir.AluOpType.mult)
                    ob = pool.tile([128, 128], BF, tag="ob")
                    nc.vector.tensor_scalar_max(ob[:], o32[:], 0.0)
                    otp = pp.tile([128, 128], BF, tag="ps")
                    nc.tensor.transpose(out=otp[:], in_=ob[:],
                                        identity=ident[:])
                    otb = pool.tile([128, 128], BF, tag="otb")
                    nc.vector.tensor_copy(out=otb[:], in_=otp[:])
                    ktp = pp.tile([128, 128], F32, tag="ps")
                    nc.tensor.matmul(out=ktp[:], lhsT=kw_t, rhs=otb[:],
                                     start=True, stop=True)
                    th = pool.tile([128, 128], F32, tag="th")
                    nc.scalar.activation(
                        out=th[:], in_=ktp[:],
                        func=mybir.ActivationFunctionType.Tanh,
                        bias=kb_t)
                    red = pool.tile([128, 1], F32, tag="red")
                    nlast = p.LAST_TILE_N if t_i == p.DST_TILES - 1 else 128
                    nc.vector.reduce_sum(red[:], th[:, 0:nlast], axis=AXX)
                    nc.vector.tensor_add(out=tsum_t[:, mi:mi + 1],
                                         in0=tsum_t[:, mi:mi + 1],
                                         in1=red[:])
                    ppm = pp.tile([2, 128], F32, tag="ps")
                    nc.tensor.matmul(out=ppm[:], lhsT=linw_t, rhs=otb[:],
                                     start=True, stop=True)
                    po = pool.tile([2, 128], F32, tag="po")
                    nc.vector.tensor_copy(out=po[:], in_=ppm[:])
                    nc.sync.dma_start(
                        out=p_out[m][:, 128 * t_i:128 * (t_i + 1)],
                        in_=po[:])
            tspm = pp.tile([2, 128], F32, tag="ps")
            nc.tensor.matmul(out=tspm[:], lhsT=tsum_t[:], rhs=identf[:],
                             start=True, stop=True)
            tss = pool.tile([2, 128], F32, tag="tss")
            nc.vector.tensor_copy(out=tss[:], in_=tspm[:])
            nc.sync.dma_start(
                out=pout[:, 2 * p.DST_TILES * 128:2 * p.DST_TILES * 128 + 128],
                in_=tss[:])

    nc.finalize()
    return nc


# ====================== host prep ======================

_TMPL = {}


def _pad_templates(p):
    if "sidx" not in _TMPL:
        NE = N_CORES * p.ECAP
        _TMPL["sidx"] = (
            p.SLOT_REAL +
            (np.arange(NE) % (p.SLOT_BANK - p.SLOT_REAL))).astype(np.int16)
        _TMPL["didx"] = np.full(NE, p.AD_PAD, np.int16)
        _TMPL["dstslot"] = np.full(N_CORES * p.ASLOTS, -1, np.int16)
        _TMPL["scat"] = np.full(N_CORES * p.ASLOTS, p.DUMP_DST, np.int16)
    return _TMPL


def prep_metapath(src, dst, p: P):
    E = src.shape[0]
    src = np.asarray(src, np.int32)
    dst = np.asarray(dst, np.int32)
    o_lo = np.argsort(dst.astype(np.uint16), kind="stable")
    hi_s = (dst >> 16).astype(np.uint8)[o_lo]
    o1 = np.concatenate([o_lo[hi_s == 0], o_lo[hi_s == 1]])
    d1 = dst[o1]
    s1 = src[o1]
    c1 = d1 // p.PART
    dloc = d1 - c1 * p.PART

    newseg = np.empty(E, bool)
    newseg[0] = True
    newseg[1:] = d1[1:] != d1[:-1]

    st = dloc // p.W                      # window within core
    win_g = c1 * p.N_WIN + st
    counts = np.bincount(win_g, minlength=N_CORES * p.N_WIN)
    assert counts.max() <= 128, counts.max()
    wstart = np.concatenate(
        [[0], np.cumsum(counts)])[:-1].astype(np.int32)
    r = np.arange(E, dtype=np.int32) - wstart[win_g]

    bank = st // p.WPB
    stb = st - bank * p.WPB
    sidx16 = stb * 128 + r                # < SLOT_REAL
    abase = np.cumsum([0] + [p.chunks_b(b) * p.ACC_CHUNK
                             for b in range(p.N_SLOT_BANK)])
    aslot = abase[bank] + stb * 128 + r   # position in accumulation space

    NSB = p.N_SRC_BANK
    srcbank = s1 // (2 * p.PAIR_BANK)
    par = s1 & 1
    grp = (bank * NSB + srcbank) * 2 + par
    key2 = (c1 * p.N_GROUP + grp).astype(np.int32)
    assert p.N_GROUP * N_CORES <= 256
    o2 = np.argsort(key2.astype(np.uint8), kind="stable")
    g2 = key2[o2]
    gcounts = np.bincount(g2, minlength=N_CORES * p.N_GROUP)
    capg_all = np.array([p.capg(g % p.N_GROUP)
                         for g in range(N_CORES * p.N_GROUP)])

    assert (gcounts <= capg_all).all(), \
        (gcounts - capg_all).max()
    cum_all = np.concatenate(
        [[0], np.cumsum(np.tile([p.capg(g) for g in range(p.N_GROUP)],
                                N_CORES))])
    gstart = np.concatenate(
        [[0], np.cumsum(gcounts)]).astype(np.int32)
    rank = np.arange(E, dtype=np.int32) - gstart[g2]
    eidx = cum_all[g2].astype(np.int64) + rank

    NE = N_CORES * p.ECAP
    tmpl = _pad_templates(p)
    gidx = np.zeros(NE, np.int16)
    didx = tmpl["didx"].copy()
    sidx = tmpl["sidx"].copy()
    gidx[eidx] = ((s1[o2] >> 1) - srcbank[o2] * p.PAIR_BANK).astype(np.int16)
    didx[eidx] = dloc[o2].astype(np.int16)
    sidx[eidx] = sidx16[o2].astype(np.int16)

    def wrap16(a):  # [NCORES, N] -> [NCORES, 16, N//16]
        return np.ascontiguousarray(
            a.reshape(N_CORES, -1, 16).transpose(0, 2, 1))

    eidx = np.concatenate([wrap16(gidx.reshape(N_CORES, -1)),
                           wrap16(didx.reshape(N_CORES, -1)),
                           wrap16(sidx.reshape(N_CORES, -1))], axis=2)

    gaslot = c1.astype(np.int64) * p.ASLOTS + aslot
    dstslot = tmpl["dstslot"].copy()
    dstslot[gaslot] = dloc.astype(np.int16)
    scat = tmpl["scat"].copy()
    scat[gaslot[newseg]] = dloc[newseg].astype(np.int16)
    scatw = wrap16(scat.reshape(N_CORES, -1))
    drow = dstslot.reshape(N_CORES, p.ASLOTS // p.ACC_CHUNK, p.ACC_CHUNK)
    return dict(eidx=eidx, scatidx=scatw, dstrow=drow)


def prep_all(inputs, p: P, skip_x=False):
    e_cs = np.asarray(inputs["edge_cs"])
    e_ss = np.asarray(inputs["edge_ss"])
    W_s = np.asarray(inputs["W_subj"], np.float32)
    W_c = np.asarray(inputs["W_chan"], np.float32)

    mp_prep = {
        "cs": prep_metapath(e_cs[0], e_cs[1], p),
        "ss": prep_metapath(e_ss[0], e_ss[1], p),
    }
    if not skip_x:
        xs_bf = np.asarray(inputs["x_subject"], np.float32).astype(bf16)
        xc_bf = np.asarray(inputs["x_channel"], np.float32).astype(bf16)

    wcat = {}
    for m, W, att_s in (("cs", W_c, inputs["att_src_cs"]),
                        ("ss", W_s, inputs["att_src_ss"])):
        wa = np.einsum("khd,hd->kh", W.reshape(DIN, H, DH),
                       np.asarray(att_s, np.float32))
        cat = np.concatenate([W, wa], axis=1)
        for parn in range(2):
            z = np.zeros((128, 136), np.float32)
            z[64 * parn:64 * (parn + 1), :] = cat
            wcat[(m, parn)] = z.astype(bf16)
    wad = np.concatenate(
        [np.einsum("khd,hd->kh", W_s.reshape(DIN, H, DH),
                   np.asarray(inputs["att_dst_cs"], np.float32)),
         np.einsum("khd,hd->kh", W_s.reshape(DIN, H, DH),
                   np.asarray(inputs["att_dst_ss"], np.float32))],
        axis=1).astype(bf16)
    kw = np.asarray(inputs["k_w"], np.float32).astype(bf16)
    kb = np.ascontiguousarray(
        np.asarray(inputs["k_b"], np.float32).reshape(128, 1))
    linw = np.asarray(inputs["lin_w"], np.float32).astype(bf16)

    in_maps = []
    for c in range(N_CORES):
        im = {
            "wad": wad, "kw": kw, "kb": kb, "linw": linw,
        }
        if not skip_x:
            im["xs_sh"] = np.ascontiguousarray(
                xs_bf[c * p.PART:(c + 1) * p.PART])
            im["xc_sh"] = np.ascontiguousarray(
                xc_bf[c * p.PART:(c + 1) * p.PART])
        for m in MP:
            pr = mp_prep[m]
            for nm in ("gidx", "didx", "sidx", "scatidx", "dstrow"):
                im[f"{nm}_{m}"] = pr[nm][c]
            for parn in range(2):
                im[f"wcat_{m}{parn}"] = wcat[(m, parn)]
        in_maps.append(im)
    return in_maps


def epilogue(results, inputs, p: P):
    NS = p.NNODE
    q = np.asarray(inputs["q"], np.float32)
    lin_b = np.asarray(inputs["lin_b"], np.float32)
    D2 = 2 * p.DST_TILES * 128
    tsum = sum(np.asarray(r["pout"], np.float32)[:, D2:D2 + 128].T
               for r in results)
    score = (tsum / np.float32(NS)).T @ q
    score = score - score.max()
    attn = np.exp(score)
    attn /= attn.sum()
    D = p.DST_TILES * 128
    out = np.empty((NS, 2), np.float32)
    for c, r in enumerate(results):
        po = np.asarray(r["pout"], np.float32)
        pc = po[:, :p.PART]
        ps = po[:, D:D + p.PART]
        out[c * p.PART:(c + 1) * p.PART] = (attn[0] * pc + attn[1] * ps).T
    return out + lin_b


# ====================== persistent runner ======================

_RT = None


def _make_runner(nc):
    import jax
    import jax.core
    from jax.sharding import Mesh, PartitionSpec
    from jax.experimental.shard_map import shard_map
    from concourse.bass2jax import (
        _bass_exec_p, install_neuronx_cc_hook, partition_id_tensor)

    install_neuronx_cc_hook()
    part_name = (nc.partition_id_tensor.name
                 if nc.partition_id_tensor else None)
    in_names, out_names, out_avals = [], [], []
    for alloc in nc.m.functions[0].allocations:
        if not isinstance(alloc, mybir.MemoryLocationSet):
            continue
        name = alloc.memorylocations[0].name
        if alloc.kind == "ExternalInput":
            if name != part_name:
                in_names.append(name)
        elif alloc.kind == "ExternalOutput":
            out_names.append(name)
            out_avals.append(jax.core.ShapedArray(
                tuple(alloc.tensor_shape), mybir.dt.np(alloc.dtype)))
    n_params = len(in_names)
    donate = tuple(range(n_params, n_params + len(out_names)))

    bind_names = in_names + out_names + ([part_name] if part_name else [])

    def _body(*args):
        ops = list(args)
        if part_name is not None:
            ops.append(partition_id_tensor())
        return tuple(_bass_exec_p.bind(
            *ops, out_avals=tuple(out_avals),
            in_names=tuple(bind_names),
            out_names=tuple(out_names),
            lowering_input_output_aliases=(),
            sim_require_finite=False, sim_require_nnan=False, nc=nc))

    devices = jax.devices()[:N_CORES]
    assert len(devices) >= N_CORES
    mesh = Mesh(np.asarray(devices[:N_CORES]), ("core",))
    nio = n_params + len(out_names)
    sharded = jax.jit(
        shard_map(_body, mesh=mesh,
                  in_specs=(PartitionSpec("core"),) * nio,
                  out_specs=(PartitionSpec("core"),) * len(out_names),
                  check_rep=False),
        donate_argnums=donate, keep_unused=True)
    from jax.sharding import NamedSharding
    shd = NamedSharding(mesh, PartitionSpec("core"))

    import jax.numpy as jnp

    @jax.jit
    def _mkzeros():
        return tuple(
            jnp.zeros((N_CORES * a.shape[0], *a.shape[1:]), a.dtype)
            for a in out_avals)

    mkzeros = jax.jit(_mkzeros, out_shardings=(shd,) * len(out_avals))
    return sharded, in_names, out_names, out_avals, shd, mkzeros


def _ensure_runtime():
    global _RT
    if _RT is None:
        nc = build(FULL)
        _RT = _make_runner(nc)
    return _RT


def _run_device(in_maps):
    sharded, in_names, out_names, out_avals, _, _mz = _ensure_runtime()
    concat_in = [np.concatenate([in_maps[c][nm] for c in range(N_CORES)],
                                axis=0) for nm in in_names]
    concat_zeros = [np.zeros((N_CORES * a.shape[0], *a.shape[1:]), a.dtype)
                    for a in out_avals]
    outs = sharded(*concat_in, *concat_zeros)
    results = []
    for c in range(N_CORES):
        results.append({nm: np.asarray(o).reshape(
            N_CORES, *out_avals[i].shape)[c]
            for i, (nm, o) in enumerate(zip(out_names, outs))})
    return results


def _input_shapes(p):
    sh = {
        "xs_sh": ((p.PART, DIN), np.float16),
        "xc_sh": ((p.PART, DIN), np.float16),
        "wad": ((DIN, 16), np.float16),
        "kw": ((128, 128), np.float16),
        "kb": ((128, 1), np.float32),
        "linw": ((128, 2), np.float16),
    }
    for m in MP:
        sh[f"gidx_{m}"] = ((16, p.ECAP // 16), np.int16)
        sh[f"didx_{m}"] = ((16, p.ECAP // 16), np.int16)
        sh[f"sidx_{m}"] = ((16, p.ECAP // 16), np.int16)
        sh[f"scatidx_{m}"] = ((16, p.ASLOTS // 16), np.int16)
        sh[f"dstrow_{m}"] = ((p.ASLOTS // p.ACC_CHUNK, p.ACC_CHUNK), np.int16)
        for parn in range(2):
            sh[f"wcat_{m}{parn}"] = ((128, 136), np.float16)
    return sh


def _warmup():
    """Compile + NEFF load + one dummy end-to-end call at import time."""
    p = FULL
    ar = np.arange(500000, dtype=np.int64)
    ed = (ar % p.NNODE).astype(np.int32)[None, :]
    es = ((ar * 40503 + 12345) % p.NNODE).astype(np.int32)[None, :]
    dummy = {
        "x_subject": np.zeros((p.NNODE, DIN), np.float32),
        "x_channel": np.zeros((p.NNODE, DIN), np.float32),
        "edge_cs": np.concatenate([es, ed]), "edge_ss": np.concatenate([es, ed]),
        "W_subj": np.zeros((DIN, 128), np.float32),
        "b_subj": np.zeros(128, np.float32),
        "W_chan": np.zeros((DIN, 128), np.float32),
        "b_chan": np.zeros(128, np.float32),
        "att_src_cs": np.zeros((H, DH), np.float32),
        "att_dst_cs": np.zeros((H, DH), np.float32),
        "att_src_ss": np.zeros((H, DH), np.float32),
        "att_dst_ss": np.zeros((H, DH), np.float32),
        "k_w": np.zeros((128, 128), np.float32),
        "k_b": np.zeros(128, np.float32),
        "q": np.zeros(128, np.float32),
        "lin_w": np.zeros((128, 2), np.float32),
        "lin_b": np.zeros(2, np.float32),
    }
    kernel(**dummy)


def _consts_blob(inputs):
    """[128, 691] f16: 4x wcat(136) | wad(16, rows 0:64) | kw(128) |
    linw(2) | kb(1)."""
    W_s = np.asarray(inputs["W_subj"], np.float32)
    W_c = np.asarray(inputs["W_chan"], np.float32)
    blob = np.zeros((128, 691), np.float32)
    for mi, (Wm, att_s) in enumerate(((W_c, inputs["att_src_cs"]),
                                      (W_s, inputs["att_src_ss"]))):
        wa = np.einsum("khd,hd->kh", Wm.reshape(DIN, H, DH),
                       np.asarray(att_s, np.float32))
        cat = np.concatenate([Wm, wa], axis=1)
        for parn in range(2):
            off = (mi * 2 + parn) * 136
            blob[64 * parn:64 * (parn + 1), off:off + 136] = cat
    blob[0:DIN, 544:560] = np.concatenate(
        [np.einsum("khd,hd->kh", W_s.reshape(DIN, H, DH),
                   np.asarray(inputs["att_dst_cs"], np.float32)),
         np.einsum("khd,hd->kh", W_s.reshape(DIN, H, DH),
                   np.asarray(inputs["att_dst_ss"], np.float32))],
        axis=1)
    blob[:, 560:688] = np.asarray(inputs["k_w"], np.float32)
    blob[:, 688:690] = np.asarray(inputs["lin_w"], np.float32)
    blob[:, 690] = np.asarray(inputs["k_b"], np.float32)
    return blob.astype(np.float16)


def kernel(**inputs):
    import jax
    p = FULL
    sharded, in_names, out_names, out_avals, shd, mkzeros = _ensure_runtime()
    # start the big x transfers first (async), overlap with host prep
    xs16 = np.asarray(np.asarray(inputs["x_subject"], np.float32)
                      .astype(np.float16))
    xc16 = np.asarray(np.asarray(inputs["x_channel"], np.float32)
                      .astype(np.float16))
    dev = {"xs_sh": jax.device_put(xs16, shd),
           "xc_sh": jax.device_put(xc16, shd)}
    # per-metapath prep; upload each metapath's arrays as soon as ready so
    # the second metapath's host prep overlaps the first one's transfer
    edges = {"cs": np.asarray(inputs["edge_cs"]),
             "ss": np.asarray(inputs["edge_ss"])}
    for m in MP:
        pr = prep_metapath(edges[m][0], edges[m][1], p)
        for nm in ("eidx", "scatidx", "dstrow"):
            a = pr[nm]
            dev[f"{nm}_{m}"] = jax.device_put(
                np.ascontiguousarray(a.reshape(-1, a.shape[-1])), shd)
    consts = {"cblob": _consts_blob(inputs)}
    args = []
    for nm in in_names:
        if nm in dev:
            args.append(dev[nm])
        else:
            args.append(np.concatenate([consts[nm]] * N_CORES, axis=0))
    zeros = mkzeros()
    outs = sharded(*args, *zeros)
    fetched = jax.device_get(list(outs))
    results = []
    for c in range(N_CORES):
        results.append({nm: fetched[i].reshape(
            N_CORES, *out_avals[i].shape)[c]
            for i, nm in enumerate(out_names)})
    return epilogue(results, inputs, p)


_warmup()


# revision 16
# speedup vs baseline: 1.0343x; 1.0343x over previous
"""HAN (heterogeneous graph attention) kernel for nn_BNNHAN_11038065951338.

Runs on 8 Trainium2 NeuronCores via Bass (compiled once at import, executed
through a persistent jax/PJRT executable). kernel(**inputs) takes the FULL
unsharded numpy inputs and returns the FULL [100000, 2] float32 output.

Per core (dst-partitioned, 1/8 of subject nodes):
  P0: AllGather bf16 x tables; build a_dst table; memset intermediates.
  P1: edge phase — per (slotbank, srcbank, parity) group of edges:
      pair-transpose-gather x_src, gather a_dst, project+attend on PE,
      exp(lrelu) on ACT, scatter msg||exp to a unique dst-sorted slot.
  P2: accumulation — read slots in dst order, dedup each 128-slot tile
      with a self-match matrix matmul, scatter one row per dst segment.
  P3: normalize (softmax denom), relu, semantic-attention partials and
      2-dim output projections.
Host: bucket/sort edges, assemble int16 index arrays; final 2-way
metapath softmax combine.
"""

import dataclasses
import time

import numpy as np
import ml_dtypes

import jax as _jax
try:
    _jax.config.update("jax_compilation_cache_dir", "/tmp/jax_cache_han")
    _jax.config.update("jax_persistent_cache_min_entry_size_bytes", -1)
    _jax.config.update("jax_persistent_cache_min_compile_time_secs", 0.0)
except Exception:
    pass

import concourse.bass as bass
import concourse.bacc as bacc
import concourse.mybir as mybir
import concourse.tile as tile
import bass_rust
from concourse.masks import make_identity

bf16 = np.float16
F32 = mybir.dt.float32
BF = mybir.dt.float16
I16 = mybir.dt.int16
AXX = bass_rust.AxisListType.X
N_CORES = 8
H, DH, F = 8, 16, 128
DIN = 64
MP = ("cs", "ss")


@dataclasses.dataclass
class P:
    NNODE: int = 100000          # nodes per type
    PART: int = 12500            # dst nodes per core
    PAIR_BANK: int = 16384       # gather idx per src bank (pairs)
    W: int = 16                  # dst nodes per 128-slot window
    WPB: int = 224               # windows per slot bank
    SLOT_BANK: int = 32768       # slot-idx space per scatter bank
    ACC_CHUNK: int = 4096        # slots per accumulation chunk
    CAPS_TBL: tuple = ((3328, 3328, 3328, 256), (3328, 3328, 3328, 256),
                       (3328, 3328, 3328, 256), (1792, 1792, 1792, 256))

    @property
    def N_SRC_BANK(self):
        return max(1, -(-(self.NNODE // 2) // self.PAIR_BANK))

    @property
    def N_WIN(self):
        return -(-self.PART // self.W)

    @property
    def N_SLOT_BANK(self):
        return -(-self.N_WIN // self.WPB)

    @property
    def SLOT_REAL(self):
        return self.WPB * 128

    def wb(self, b):
        return min(self.WPB, self.N_WIN - self.WPB * b)

    def chunks_b(self, b):
        return -(-self.wb(b) * 128 // self.ACC_CHUNK)

    @property
    def ACC_REGIONS(self):
        return [(b, ch) for b in range(self.N_SLOT_BANK)
                for ch in range(self.chunks_b(b))]

    @property
    def ASLOTS(self):
        return len(self.ACC_REGIONS) * self.ACC_CHUNK

    @property
    def N_GROUP(self):
        return self.N_SLOT_BANK * self.N_SRC_BANK * 2

    def capg(self, g):
        return self.CAPS_TBL[g // (self.N_SRC_BANK * 2)][
            (g // 2) % self.N_SRC_BANK]

    @property
    def CUMCAPS(self):
        import itertools
        return [0] + list(itertools.accumulate(
            self.capg(g) for g in range(self.N_GROUP)))

    @property
    def ECAP(self):
        return self.CUMCAPS[-1]

    @property
    def ACC_SUB(self):
        return self.ACC_CHUNK // 128

    @property
    def DST_TILES(self):
        return (self.PART + 127) // 128

    @property
    def LAST_TILE_N(self):
        return self.PART - (self.DST_TILES - 1) * 128

    @property
    def FT_ROWS(self):
        return self.DST_TILES * 128 + 128

    @property
    def DUMP_DST(self):
        return self.DST_TILES * 128 + 9

    @property
    def AD_PAD(self):
        return self.PART + 8


FULL = P()
SMALL = P(NNODE=2048, PART=256, PAIR_BANK=1024, W=16, WPB=6,
          SLOT_BANK=1024, ACC_CHUNK=1024,
          CAPS_TBL=((384,), (384,), (256,)))


def build(p: P, phases: int = 3, p1mode: int = 0):
    nc = bacc.Bacc("TRN2", target_bir_lowering=False, debug=False,
                   num_devices=N_CORES)
    NPAIR = p.NNODE // 2

    xs_sh = nc.dram_tensor("xs_sh", [p.PART, DIN], BF, kind="ExternalInput")
    xc_sh = nc.dram_tensor("xc_sh", [p.PART, DIN], BF, kind="ExternalInput")
    E16 = p.ECAP // 16
    ins = {}
    for m in MP:
        ins[f"eidx_{m}"] = nc.dram_tensor(f"eidx_{m}", [16, 3 * E16],
                                          I16, kind="ExternalInput")
        ins[f"scatidx_{m}"] = nc.dram_tensor(
            f"scatidx_{m}", [16, p.ASLOTS // 16], I16, kind="ExternalInput")
        ins[f"dstrow_{m}"] = nc.dram_tensor(
            f"dstrow_{m}", [p.ASLOTS // p.ACC_CHUNK, p.ACC_CHUNK], I16,
            kind="ExternalInput")
    cblob = nc.dram_tensor("cblob", [128, 691], BF, kind="ExternalInput")

    pout = nc.dram_tensor("pout", [2, 2 * p.DST_TILES * 128 + 128],
                          F32, kind="ExternalOutput")
    p_out = {m: pout[:, mi * p.DST_TILES * 128:
                     (mi + 1) * p.DST_TILES * 128]
             for mi, m in enumerate(MP)}

    xs_b = nc.dram_tensor("xs_b", [p.PART, DIN], BF)
    xc_b = nc.dram_tensor("xc_b", [p.PART, DIN], BF)
    xs_full = nc.dram_tensor("xs_full", [p.NNODE, DIN], BF,
                             addr_space="Shared")
    xc_full = nc.dram_tensor("xc_full", [p.NNODE, DIN], BF,
                             addr_space="Shared")
    ad_tab = nc.dram_tensor("ad_tab", [p.PART + 64, DIN], F32)
    inter = {m: nc.dram_tensor(f"inter_{m}", [p.N_SLOT_BANK * p.SLOT_BANK, 256], BF) for m in MP}
    ftab = {m: nc.dram_tensor(f"ftab_{m}", [p.FT_ROWS, 256], BF) for m in MP}
    rep = {}
    for m in MP:
        for nm, ncol in (("eidx", 3 * E16), ("scatidx", p.ASLOTS // 16)):
            rep[f"{nm}_{m}"] = nc.dram_tensor(f"r{nm}_{m}", [128, ncol], I16)

    x_full = {"cs": xc_full, "ss": xs_full}

    with tile.TileContext(nc) as tc:
        with (
            tc.tile_pool(name="consts", bufs=1) as cpool,
            tc.tile_pool(name="sbuf", bufs=2) as pool,
            tc.tile_pool(name="acc1", bufs=1) as apool,
            tc.tile_pool(name="psum", bufs=8, space="PSUM") as pp,
        ):
            # ================= P0 =================
            nc.sync.dma_start(out=xs_b[:], in_=xs_sh[:])
            nc.sync.dma_start(out=xc_b[:], in_=xc_sh[:])
            nc.gpsimd.collective_compute(
                "AllGather", mybir.AluOpType.bypass,
                replica_groups=[list(range(N_CORES))],
                ins=[xs_b[:]], outs=[xs_full[:]])
            nc.gpsimd.collective_compute(
                "AllGather", mybir.AluOpType.bypass,
                replica_groups=[list(range(N_CORES))],
                ins=[xc_b[:]], outs=[xc_full[:]])

            for m in MP:
                for nm in ("eidx", "scatidx"):
                    for k in range(8):
                        nc.sync.dma_start(
                            out=rep[f"{nm}_{m}"][16 * k:16 * (k + 1), :],
                            in_=ins[f"{nm}_{m}"][:])

            ident = cpool.tile([128, 128], BF)
            make_identity(nc, ident[:])
            identf = cpool.tile([128, 128], F32)
            make_identity(nc, identf[:])
            cb = cpool.tile([128, 691], BF)
            nc.sync.dma_start(out=cb[:], in_=cblob[:])
            wcat_t = {}
            for mi_, m in enumerate(MP):
                for par in range(2):
                    off = (mi_ * 2 + par) * 136
                    wcat_t[(m, par)] = cb[:, off:off + 136]
            wad_t = cb[0:DIN, 544:560]
            kw_t = cb[:, 560:688]
            linw_t = cb[:, 688:690]
            kb_t = cb[:, 690:691]

            # a_d table
            for t_i in range(p.DST_TILES):
                lo = t_i * 128
                n = min(128, p.PART - lo)
                xst = pool.tile([128, DIN], BF, tag="xst")
                if n < 128:
                    nc.vector.memset(xst[:], 0.0)
                nc.sync.dma_start(out=xst[:n, :], in_=xs_sh[lo:lo + n, :])
                tp = pp.tile([DIN, 128], BF, tag="ps")
                nc.tensor.transpose(out=tp[:], in_=xst[:], identity=ident[:])
                xstT = pool.tile([DIN, 128], BF, tag="xstT")
                nc.vector.tensor_copy(out=xstT[:], in_=tp[:])
                adp = pp.tile([128, 16], F32, tag="ps")
                nc.tensor.matmul(out=adp[:], lhsT=xstT[:], rhs=wad_t,
                                 start=True, stop=True)
                ads = pool.tile([128, 16], F32, tag="ads")
                nc.vector.tensor_copy(out=ads[:], in_=adp[:])
                nc.sync.dma_start(out=ad_tab[lo:lo + n, 0:16], in_=ads[:n, :])
            padt = cpool.tile([128, DIN], F32)
            nc.vector.memset(padt[:], -100.0)
            nc.sync.dma_start(out=ad_tab[p.PART:p.PART + 64, :],
                              in_=padt[0:64, :])

            zt = cpool.tile([128, 16, 256], BF)
            nc.vector.memset(zt[:], 0.0)
            for m in MP:
                iv = inter[m][:].rearrange("(a q) c -> q a c", q=128)
                for b in range(p.N_SLOT_BANK):
                    base = b * p.SLOT_BANK // 128
                    nreg = p.chunks_b(b) * p.ACC_CHUNK // 128
                    ndmp = (p.SLOT_BANK - p.SLOT_REAL) // 128
                    for k0, k1 in ((base, base + nreg),
                                   (base + p.SLOT_REAL // 128,
                                    base + p.SLOT_REAL // 128 + ndmp)):
                        for k in range(k0, k1, 16):
                            w = min(16, k1 - k)
                            nc.sync.dma_start(out=iv[:, k:k + w, :],
                                              in_=zt[:, :w, :])
                fv = ftab[m][:].rearrange("(a q) c -> q a c", q=128)
                nfa = p.FT_ROWS // 128
                for k in range(0, nfa, 16):
                    w = min(16, nfa - k)
                    nc.sync.dma_start(out=fv[:, k:k + w, :],
                                      in_=zt[:, :w, :])

            # ================= P1: edge phase =================
            for m in (MP if phases >= 1 else ()):
                NSB = p.N_SRC_BANK
                for g in range(p.N_GROUP):
                    sb_bank = g // (2 * NSB)
                    srcb = (g // 2) % NSB
                    par = g % 2
                    CAP = p.capg(g)
                    EPS = CAP // 128
                    c0 = p.CUMCAPS[g] // 16
                    ncol = CAP // 16
                    E16_ = p.ECAP // 16
                    gi = pool.tile([128, ncol], I16, tag="gi")
                    nc.sync.dma_start(
                        out=gi[:], in_=rep[f"eidx_{m}"][:, c0:c0 + ncol])
                    di = pool.tile([128, ncol], I16, tag="di")
                    nc.sync.dma_start(
                        out=di[:],
                        in_=rep[f"eidx_{m}"][:, E16_ + c0:E16_ + c0 + ncol])
                    si = pool.tile([128, ncol], I16, tag="si")
                    nc.sync.dma_start(
                        out=si[:],
                        in_=rep[f"eidx_{m}"][:, 2 * E16_ + c0:
                                             2 * E16_ + c0 + ncol])

                    xg = pool.tile([128, EPS, 128], BF, tag="xg")
                    bank_ap = bass.AP(
                        x_full[m][:].tensor, srcb * p.PAIR_BANK * 128,
                        [[128, NPAIR - srcb * p.PAIR_BANK], [1, 128]])
                    if p1mode < 2:
                        nc.gpsimd.dma_gather(
                            xg[:], bank_ap, gi[:], CAP, CAP, 128,
                            single_packet=False)
                    else:
                        nc.vector.memset(xg[:], 0.25)
                    ad = pool.tile([128, EPS, DIN], F32, tag="ad")
                    if p1mode < 1:
                        nc.gpsimd.dma_gather(
                            ad[:], ad_tab[:], di[:], CAP, CAP, DIN,
                            single_packet=False)
                    else:
                        nc.vector.memset(ad[:], 0.25)

                    talpha = pool.tile([128, EPS * 8], F32, tag="ta")
                    etile = pool.tile([128, EPS * 8], F32, tag="et")
                    scb = pool.tile([128, EPS, 256], BF, tag="scb")
                    adoff = 0 if m == "cs" else 8
                    GB = 4  # psum group batch
                    for s0 in range(0, EPS, GB):
                        s1 = min(s0 + GB, EPS)
                        hps = []
                        for s in range(s0, s1):
                            xtp = pp.tile([128, 128], BF, tag="ps")
                            nc.tensor.transpose(out=xtp[:], in_=xg[:, s, :],
                                                identity=ident[:])
                            xts = pool.tile([128, 128], BF, tag="xts")
                            nc.vector.tensor_copy(out=xts[:], in_=xtp[:])
                            hp = pp.tile([128, 136], F32, tag="ps")
                            nc.tensor.matmul(
                                out=hp[:], lhsT=xts[:],
                                rhs=wcat_t[(m, par)],
                                start=True, stop=True)
                            nc.vector.tensor_tensor(
                                out=talpha[:, 8 * s:8 * (s + 1)],
                                in0=hp[:, 128:136],
                                in1=ad[:, s, adoff:adoff + 8],
                                op=mybir.AluOpType.add)
                            hps.append(hp)
                        nsub = s1 - s0
                        tsl = talpha[:, 8 * s0:8 * s1]
                        usl = etile[:, 8 * s0:8 * s1]
                        nc.vector.tensor_scalar_mul(usl, tsl, 0.2)
                        nc.vector.tensor_tensor(out=tsl, in0=tsl, in1=usl,
                                                op=mybir.AluOpType.max)
                        nc.scalar.activation(
                            out=usl, in_=tsl,
                            func=mybir.ActivationFunctionType.Exp)
                        esl = scb[:, s0, 128:136]
                        eout = bass.AP(esl.tensor, esl.offset,
                                       [esl.ap[0], [256, nsub], [1, 8]])
                        nc.vector.tensor_copy(out=eout,
                                              in_=etile[:, 8 * s0:8 * s1])
                        for s in range(s0, s1):
                            ea = etile[:, 8 * s:8 * s + 8]
                            ebc = bass.AP(ea.tensor, ea.offset,
                                          [ea.ap[0], [1, 8], [0, 16]])
                            nc.vector.tensor_tensor(
                                out=scb[:, s, 0:128],
                                in0=hps[s - s0][:, 0:128], in1=ebc,
                                op=mybir.AluOpType.mult)
                    if p1mode < 3:
                        nc.gpsimd.dma_scatter_add(
                            inter[m][sb_bank * p.SLOT_BANK:
                                     (sb_bank + 1) * p.SLOT_BANK, :],
                            scb[:], si[:], CAP, CAP, 256,
                            single_packet=False)

            # ================= P2: accumulation =================
            for m in (MP if phases >= 2 else ()):
                dcol_i = apool.tile([128, p.ASLOTS // 128], I16,
                                    tag=f"dcoli{m}")
                nc.sync.dma_start(
                    out=dcol_i[:],
                    in_=ins[f"dstrow_{m}"][:].rearrange(
                        "a (b c) -> (a b) c", c=128),
                    transpose=True)
                for ch, (rb, rch) in enumerate(p.ACC_REGIONS):
                    a0 = (rb * p.SLOT_BANK + rch * p.ACC_CHUNK) // 128
                    me = pool.tile([128, p.ACC_SUB, 256], BF, tag="me")
                    nc.sync.dma_start(
                        out=me[:],
                        in_=inter[m][:].rearrange("(a q) c -> q a c", q=128)[
                            :, a0:a0 + p.ACC_SUB, :])
                    sci = pool.tile([128, p.ACC_CHUNK // 16], I16, tag="sci")
                    nc.sync.dma_start(
                        out=sci[:],
                        in_=rep[f"scatidx_{m}"][
                            :, ch * (p.ACC_CHUNK // 16):
                            (ch + 1) * (p.ACC_CHUNK // 16)])
                    rt = pool.tile([128, p.ACC_CHUNK], I16, tag="rt")
                    rsrc = bass.AP(ins[f"dstrow_{m}"][:].tensor,
                                   ch * p.ACC_CHUNK,
                                   [[0, 128], [1, p.ACC_CHUNK]])
                    nc.sync.dma_start(out=rt[:], in_=rsrc)
                    scat = pool.tile([128, p.ACC_SUB, 256], BF, tag="scat")
                    for s in range(p.ACC_SUB):
                        st = ch * p.ACC_SUB + s
                        csl = dcol_i[:, st:st + 1]
                        col_bc = bass.AP(csl.tensor, csl.offset,
                                         [csl.ap[0], [0, 128]])
                        S = pool.tile([128, 128], BF, tag="S")
                        nc.vector.tensor_tensor(
                            out=S[:], in0=col_bc,
                            in1=rt[:, 128 * s:128 * (s + 1)],
                            op=mybir.AluOpType.is_equal)
                        dp = pp.tile([128, 136], F32, tag="ps")
                        nc.tensor.matmul(out=dp[:], lhsT=S[:],
                                         rhs=me[:, s, 0:136],
                                         start=True, stop=True)
                        nc.vector.tensor_copy(out=scat[:, s, 0:136],
                                              in_=dp[:])
                    nc.gpsimd.dma_scatter_add(
                        ftab[m][:], scat[:], sci[:],
                        p.ACC_CHUNK, p.ACC_CHUNK, 256,
                        single_packet=False)

            # ================= P3: epilogue =================
            tsum_t = apool.tile([128, 2], F32, tag="tsum")
            nc.vector.memset(tsum_t[:], 0.0)
            for mi, m in enumerate(MP if phases >= 3 else ()):
                for t_i in range(p.DST_TILES):
                    ft = pool.tile([128, 136], BF, tag="ft")
                    nc.sync.dma_start(
                        out=ft[:],
                        in_=ftab[m][:].rearrange("(a q) c -> q a c", q=128)[
                            :, t_i, 0:136])
                    sden = pool.tile([128, 8], F32, tag="sden")
                    nc.vector.tensor_scalar_add(sden[:], ft[:, 128:136],
                                                1e-16)
                    rec = pool.tile([128, 8], F32, tag="rec")
                    nc.vector.reciprocal(rec[:], sden[:])
                    o32 = pool.tile([128, 128], F32, tag="o32")
                    rbc = bass.AP(rec.tensor, rec[:].offset,
                                  [rec[:].ap[0], [1, 8], [0, 16]])
                    nc.vector.tensor_tensor(out=o32[:], in0=ft[:, 0:128],
                                            in1=rbc,
                                            op=mybir.AluOpType.mult)
                    ob = pool.tile([128, 128], BF, tag="ob")
                    nc.vector.tensor_scalar_max(ob[:], o32[:], 0.0)
                    otp = pp.tile([128, 128], BF, tag="ps")
                    nc.tensor.transpose(out=otp[:], in_=ob[:],
                                        identity=ident[:])
                    otb = pool.tile([128, 128], BF, tag="otb")
                    nc.vector.tensor_copy(out=otb[:], in_=otp[:])
                    ktp = pp.tile([128, 128], F32, tag="ps")
                    nc.tensor.matmul(out=ktp[:], lhsT=kw_t, rhs=otb[:],
                                     start=True, stop=True)
                    th = pool.tile([128, 128], F32, tag="th")
                    nc.scalar.activation(
                        out=th[:], in_=ktp[:],
                        func=mybir.ActivationFunctionType.Tanh,
                        bias=kb_t)
                    red = pool.tile([128, 1], F32, tag="red")
                    nlast = p.LAST_TILE_N if t_i == p.DST_TILES - 1 else 128
                    nc.vector.reduce_sum(red[:], th[:, 0:nlast], axis=AXX)
                    nc.vector.tensor_add(out=tsum_t[:, mi:mi + 1],
                                         in0=tsum_t[:, mi:mi + 1],
                                         in1=red[:])
                    ppm = pp.tile([2, 128], F32, tag="ps")
                    nc.tensor.matmul(out=ppm[:], lhsT=linw_t, rhs=otb[:],
                                     start=True, stop=True)
                    po = pool.tile([2, 128], F32, tag="po")
                    nc.vector.tensor_copy(out=po[:], in_=ppm[:])
                    nc.sync.dma_start(
                        out=p_out[m][:, 128 * t_i:128 * (t_i + 1)],
                        in_=po[:])
            tspm = pp.tile([2, 128], F32, tag="ps")
            nc.tensor.matmul(out=tspm[:], lhsT=tsum_t[:], rhs=identf[:],
                             start=True, stop=True)
            tss = pool.tile([2, 128], F32, tag="tss")
            nc.vector.tensor_copy(out=tss[:], in_=tspm[:])
            nc.sync.dma_start(
                out=pout[:, 2 * p.DST_TILES * 128:2 * p.DST_TILES * 128 + 128],
                in_=tss[:])

    nc.finalize()
    return nc


# ====================== host prep ======================

_TMPL = {}


def _pad_templates(p):
    if "sidx" not in _TMPL:
        NE = N_CORES * p.ECAP
        _TMPL["sidx"] = (
            p.SLOT_REAL +
            (np.arange(NE) % (p.SLOT_BANK - p.SLOT_REAL))).astype(np.int16)
        _TMPL["didx"] = np.full(NE, p.AD_PAD, np.int16)
        _TMPL["dstslot"] = np.full(N_CORES * p.ASLOTS, -1, np.int16)
        _TMPL["scat"] = np.full(N_CORES * p.ASLOTS, p.DUMP_DST, np.int16)
    return _TMPL


def prep_metapath(src, dst, p: P):
    E = src.shape[0]
    src = np.asarray(src, np.int32)
    dst = np.asarray(dst, np.int32)
    o_lo = np.argsort(dst.astype(np.uint16), kind="stable")
    hi_s = (dst >> 16).astype(np.uint8)[o_lo]
    o1 = np.concatenate([o_lo[hi_s == 0], o_lo[hi_s == 1]])
    d1 = dst[o1]
    s1 = src[o1]
    c1 = d1 // p.PART
    dloc = d1 - c1 * p.PART

    newseg = np.empty(E, bool)
    newseg[0] = True
    newseg[1:] = d1[1:] != d1[:-1]

    st = dloc // p.W                      # window within core
    win_g = c1 * p.N_WIN + st
    counts = np.bincount(win_g, minlength=N_CORES * p.N_WIN)
    assert counts.max() <= 128, counts.max()
    wstart = np.concatenate(
        [[0], np.cumsum(counts)])[:-1].astype(np.int32)
    r = np.arange(E, dtype=np.int32) - wstart[win_g]

    bank = st // p.WPB
    stb = st - bank * p.WPB
    sidx16 = stb * 128 + r                # < SLOT_REAL
    abase = np.cumsum([0] + [p.chunks_b(b) * p.ACC_CHUNK
                             for b in range(p.N_SLOT_BANK)])
    aslot = abase[bank] + stb * 128 + r   # position in accumulation space

    NSB = p.N_SRC_BANK
    srcbank = s1 // (2 * p.PAIR_BANK)
    par = s1 & 1
    grp = (bank * NSB + srcbank) * 2 + par
    key2 = (c1 * p.N_GROUP + grp).astype(np.int32)
    assert p.N_GROUP * N_CORES <= 256
    o2 = np.argsort(key2.astype(np.uint8), kind="stable")
    g2 = key2[o2]
    gcounts = np.bincount(g2, minlength=N_CORES * p.N_GROUP)
    capg_all = np.array([p.capg(g % p.N_GROUP)
                         for g in range(N_CORES * p.N_GROUP)])

    assert (gcounts <= capg_all).all(), \
        (gcounts - capg_all).max()
    cum_all = np.concatenate(
        [[0], np.cumsum(np.tile([p.capg(g) for g in range(p.N_GROUP)],
                                N_CORES))])
    gstart = np.concatenate(
        [[0], np.cumsum(gcounts)]).astype(np.int32)
    rank = np.arange(E, dtype=np.int32) - gstart[g2]
    eidx = cum_all[g2].astype(np.int64) + rank

    NE = N_CORES * p.ECAP
    tmpl = _pad_templates(p)
    gidx = np.zeros(NE, np.int16)
    didx = tmpl["didx"].copy()
    sidx = tmpl["sidx"].copy()
    gidx[eidx] = ((s1[o2] >> 1) - srcbank[o2] * p.PAIR_BANK).astype(np.int16)
    didx[eidx] = dloc[o2].astype(np.int16)
    sidx[eidx] = sidx16[o2].astype(np.int16)

    def wrap16(a):  # [NCORES, N] -> [NCORES, 16, N//16]
        return np.ascontiguousarray(
            a.reshape(N_CORES, -1, 16).transpose(0, 2, 1))

    eidx = np.concatenate([wrap16(gidx.reshape(N_CORES, -1)),
                           wrap16(didx.reshape(N_CORES, -1)),
                           wrap16(sidx.reshape(N_CORES, -1))], axis=2)

    gaslot = c1.astype(np.int64) * p.ASLOTS + aslot
    dstslot = tmpl["dstslot"].copy()
    dstslot[gaslot] = dloc.astype(np.int16)
    scat = tmpl["scat"].copy()
    scat[gaslot[newseg]] = dloc[newseg].astype(np.int16)
    scatw = wrap16(scat.reshape(N_CORES, -1))
    drow = dstslot.reshape(N_CORES, p.ASLOTS // p.ACC_CHUNK, p.ACC_CHUNK)
    return dict(eidx=eidx, scatidx=scatw, dstrow=drow)


def prep_all(inputs, p: P, skip_x=False):
    e_cs = np.asarray(inputs["edge_cs"])
    e_ss = np.asarray(inputs["edge_ss"])
    W_s = np.asarray(inputs["W_subj"], np.float32)
    W_c = np.asarray(inputs["W_chan"], np.float32)

    mp_prep = {
        "cs": prep_metapath(e_cs[0], e_cs[1], p),
        "ss": prep_metapath(e_ss[0], e_ss[1], p),
    }
    if not skip_x:
        xs_bf = np.asarray(inputs["x_subject"], np.float32).astype(bf16)
        xc_bf = np.asarray(inputs["x_channel"], np.float32).astype(bf16)

    wcat = {}
    for m, W, att_s in (("cs", W_c, inputs["att_src_cs"]),
                        ("ss", W_s, inputs["att_src_ss"])):
        wa = np.einsum("khd,hd->kh", W.reshape(DIN, H, DH),
                       np.asarray(att_s, np.float32))
        cat = np.concatenate([W, wa], axis=1)
        for parn in range(2):
            z = np.zeros((128, 136), np.float32)
            z[64 * parn:64 * (parn + 1), :] = cat
            wcat[(m, parn)] = z.astype(bf16)
    wad = np.concatenate(
        [np.einsum("khd,hd->kh", W_s.reshape(DIN, H, DH),
                   np.asarray(inputs["att_dst_cs"], np.float32)),
         np.einsum("khd,hd->kh", W_s.reshape(DIN, H, DH),
                   np.asarray(inputs["att_dst_ss"], np.float32))],
        axis=1).astype(bf16)
    kw = np.asarray(inputs["k_w"], np.float32).astype(bf16)
    kb = np.ascontiguousarray(
        np.asarray(inputs["k_b"], np.float32).reshape(128, 1))
    linw = np.asarray(inputs["lin_w"], np.float32).astype(bf16)

    in_maps = []
    for c in range(N_CORES):
        im = {
            "wad": wad, "kw": kw, "kb": kb, "linw": linw,
        }
        if not skip_x:
            im["xs_sh"] = np.ascontiguousarray(
                xs_bf[c * p.PART:(c + 1) * p.PART])
            im["xc_sh"] = np.ascontiguousarray(
                xc_bf[c * p.PART:(c + 1) * p.PART])
        for m in MP:
            pr = mp_prep[m]
            for nm in ("gidx", "didx", "sidx", "scatidx", "dstrow"):
                im[f"{nm}_{m}"] = pr[nm][c]
            for parn in range(2):
                im[f"wcat_{m}{parn}"] = wcat[(m, parn)]
        in_maps.append(im)
    return in_maps


def epilogue(results, inputs, p: P):
    NS = p.NNODE
    q = np.asarray(inputs["q"], np.float32)
    lin_b = np.asarray(inputs["lin_b"], np.float32)
    D2 = 2 * p.DST_TILES * 128
    tsum = sum(np.asarray(r["pout"], np.float32)[:, D2:D2 + 128].T
               for r in results)
    score = (tsum / np.float32(NS)).T @ q
    score = score - score.max()
    attn = np.exp(score)
    attn /= attn.sum()
    D = p.DST_TILES * 128
    out = np.empty((NS, 2), np.float32)
    for c, r in enumerate(results):
        po = np.asarray(r["pout"], np.float32)
        pc = po[:, :p.PART]
        ps = po[:, D:D + p.PART]
        out[c * p.PART:(c + 1) * p.PART] = (attn[0] * pc + attn[1] * ps).T
    return out + lin_b


# ====================== persistent runner ======================

_RT = None
_PREZ = None


def _make_runner(nc):
    import jax
    import jax.core
    from jax.sharding import Mesh, PartitionSpec
    from jax.experimental.shard_map import shard_map
    from concourse.bass2jax import (
        _bass_exec_p, install_neuronx_cc_hook, partition_id_tensor)

    install_neuronx_cc_hook()
    part_name = (nc.partition_id_tensor.name
                 if nc.partition_id_tensor else None)
    in_names, out_names, out_avals = [], [], []
    for alloc in nc.m.functions[0].allocations:
        if not isinstance(alloc, mybir.MemoryLocationSet):
            continue
        name = alloc.memorylocations[0].name
        if alloc.kind == "ExternalInput":
            if name != part_name:
                in_names.append(name)
        elif alloc.kind == "ExternalOutput":
            out_names.append(name)
            out_avals.append(jax.core.ShapedArray(
                tuple(alloc.tensor_shape), mybir.dt.np(alloc.dtype)))
    n_params = len(in_names)
    donate = tuple(range(n_params, n_params + len(out_names)))

    bind_names = in_names + out_names + ([part_name] if part_name else [])

    def _body(*args):
        ops = list(args)
        if part_name is not None:
            ops.append(partition_id_tensor())
        return tuple(_bass_exec_p.bind(
            *ops, out_avals=tuple(out_avals),
            in_names=tuple(bind_names),
            out_names=tuple(out_names),
            lowering_input_output_aliases=(),
            sim_require_finite=False, sim_require_nnan=False, nc=nc))

    devices = jax.devices()[:N_CORES]
    assert len(devices) >= N_CORES
    mesh = Mesh(np.asarray(devices[:N_CORES]), ("core",))
    nio = n_params + len(out_names)
    sharded = jax.jit(
        shard_map(_body, mesh=mesh,
                  in_specs=(PartitionSpec("core"),) * nio,
                  out_specs=(PartitionSpec("core"),) * len(out_names),
                  check_rep=False),
        donate_argnums=donate, keep_unused=True)
    from jax.sharding import NamedSharding
    shd = NamedSharding(mesh, PartitionSpec("core"))

    import jax.numpy as jnp

    @jax.jit
    def _mkzeros():
        return tuple(
            jnp.zeros((N_CORES * a.shape[0], *a.shape[1:]), a.dtype)
            for a in out_avals)

    mkzeros = jax.jit(_mkzeros, out_shardings=(shd,) * len(out_avals))
    return sharded, in_names, out_names, out_avals, shd, mkzeros


def _ensure_runtime():
    global _RT
    if _RT is None:
        nc = build(FULL)
        _RT = _make_runner(nc)
    return _RT


def _run_device(in_maps):
    sharded, in_names, out_names, out_avals, _, _mz = _ensure_runtime()
    concat_in = [np.concatenate([in_maps[c][nm] for c in range(N_CORES)],
                                axis=0) for nm in in_names]
    concat_zeros = [np.zeros((N_CORES * a.shape[0], *a.shape[1:]), a.dtype)
                    for a in out_avals]
    outs = sharded(*concat_in, *concat_zeros)
    results = []
    for c in range(N_CORES):
        results.append({nm: np.asarray(o).reshape(
            N_CORES, *out_avals[i].shape)[c]
            for i, (nm, o) in enumerate(zip(out_names, outs))})
    return results


def _input_shapes(p):
    sh = {
        "xs_sh": ((p.PART, DIN), np.float16),
        "xc_sh": ((p.PART, DIN), np.float16),
        "wad": ((DIN, 16), np.float16),
        "kw": ((128, 128), np.float16),
        "kb": ((128, 1), np.float32),
        "linw": ((128, 2), np.float16),
    }
    for m in MP:
        sh[f"gidx_{m}"] = ((16, p.ECAP // 16), np.int16)
        sh[f"didx_{m}"] = ((16, p.ECAP // 16), np.int16)
        sh[f"sidx_{m}"] = ((16, p.ECAP // 16), np.int16)
        sh[f"scatidx_{m}"] = ((16, p.ASLOTS // 16), np.int16)
        sh[f"dstrow_{m}"] = ((p.ASLOTS // p.ACC_CHUNK, p.ACC_CHUNK), np.int16)
        for parn in range(2):
            sh[f"wcat_{m}{parn}"] = ((128, 136), np.float16)
    return sh


def _warmup():
    """Compile + NEFF load + one dummy end-to-end call at import time."""
    p = FULL
    ar = np.arange(500000, dtype=np.int64)
    ed = (ar % p.NNODE).astype(np.int32)[None, :]
    es = ((ar * 40503 + 12345) % p.NNODE).astype(np.int32)[None, :]
    dummy = {
        "x_subject": np.zeros((p.NNODE, DIN), np.float32),
        "x_channel": np.zeros((p.NNODE, DIN), np.float32),
        "edge_cs": np.concatenate([es, ed]), "edge_ss": np.concatenate([es, ed]),
        "W_subj": np.zeros((DIN, 128), np.float32),
        "b_subj": np.zeros(128, np.float32),
        "W_chan": np.zeros((DIN, 128), np.float32),
        "b_chan": np.zeros(128, np.float32),
        "att_src_cs": np.zeros((H, DH), np.float32),
        "att_dst_cs": np.zeros((H, DH), np.float32),
        "att_src_ss": np.zeros((H, DH), np.float32),
        "att_dst_ss": np.zeros((H, DH), np.float32),
        "k_w": np.zeros((128, 128), np.float32),
        "k_b": np.zeros(128, np.float32),
        "q": np.zeros(128, np.float32),
        "lin_w": np.zeros((128, 2), np.float32),
        "lin_b": np.zeros(2, np.float32),
    }
    kernel(**dummy)


def _consts_blob(inputs):
    """[128, 691] f16: 4x wcat(136) | wad(16, rows 0:64) | kw(128) |
    linw(2) | kb(1)."""
    W_s = np.asarray(inputs["W_subj"], np.float32)
    W_c = np.asarray(inputs["W_chan"], np.float32)
    blob = np.zeros((128, 691), np.float32)
    for mi, (Wm, att_s) in enumerate(((W_c, inputs["att_src_cs"]),
                                      (W_s, inputs["att_src_ss"]))):
        wa = np.einsum("khd,hd->kh", Wm.reshape(DIN, H, DH),
                       np.asarray(att_s, np.float32))
        cat = np.concatenate([Wm, wa], axis=1)
        for parn in range(2):
            off = (mi * 2 + parn) * 136
            blob[64 * parn:64 * (parn + 1), off:off + 136] = cat
    blob[0:DIN, 544:560] = np.concatenate(
        [np.einsum("khd,hd->kh", W_s.reshape(DIN, H, DH),
                   np.asarray(inputs["att_dst_cs"], np.float32)),
         np.einsum("khd,hd->kh", W_s.reshape(DIN, H, DH),
                   np.asarray(inputs["att_dst_ss"], np.float32))],
        axis=1)
    blob[:, 560:688] = np.asarray(inputs["k_w"], np.float32)
    blob[:, 688:690] = np.asarray(inputs["lin_w"], np.float32)
    blob[:, 690] = np.asarray(inputs["k_b"], np.float32)
    return blob.astype(np.float16)


def kernel(**inputs):
    import jax
    p = FULL
    sharded, in_names, out_names, out_avals, shd, mkzeros = _ensure_runtime()
    # start the big x transfers first (async), overlap with host prep
    xs16 = np.asarray(np.asarray(inputs["x_subject"], np.float32)
                      .astype(np.float16))
    xc16 = np.asarray(np.asarray(inputs["x_channel"], np.float32)
                      .astype(np.float16))
    dev = {"xs_sh": jax.device_put(xs16, shd),
           "xc_sh": jax.device_put(xc16, shd)}
    # per-metapath prep; upload each metapath's arrays as soon as ready so
    # the second metapath's host prep overlaps the first one's transfer
    edges = {"cs": np.asarray(inputs["edge_cs"]),
             "ss": np.asarray(inputs["edge_ss"])}
    for m in MP:
        pr = prep_metapath(edges[m][0], edges[m][1], p)
        for nm in ("eidx", "scatidx", "dstrow"):
            a = pr[nm]
            dev[f"{nm}_{m}"] = jax.device_put(
                np.ascontiguousarray(a.reshape(-1, a.shape[-1])), shd)
    consts = {"cblob": _consts_blob(inputs)}
    args = []
    for nm in in_names:
        if nm in dev:
            args.append(dev[nm])
        else:
            args.append(np.concatenate([consts[nm]] * N_CORES, axis=0))
    global _PREZ
    zeros = _PREZ if _PREZ is not None else mkzeros()
    _PREZ = None
    outs = sharded(*args, *zeros)
    _PREZ = mkzeros()   # async refill for the next call
    fetched = jax.device_get(list(outs))
    results = []
    for c in range(N_CORES):
        results.append({nm: fetched[i].reshape(
            N_CORES, *out_avals[i].shape)[c]
            for i, nm in enumerate(out_names)})
    return epilogue(results, inputs, p)


_warmup()


# revision 17
# speedup vs baseline: 1.1928x; 1.1533x over previous
"""HAN (heterogeneous graph attention) kernel for nn_BNNHAN_11038065951338.

Runs on 8 Trainium2 NeuronCores via Bass (compiled once at import, executed
through a persistent jax/PJRT executable). kernel(**inputs) takes the FULL
unsharded numpy inputs and returns the FULL [100000, 2] float32 output.

Per core (dst-partitioned, 1/8 of subject nodes):
  P0: AllGather bf16 x tables; build a_dst table; memset intermediates.
  P1: edge phase — per (slotbank, srcbank, parity) group of edges:
      pair-transpose-gather x_src, gather a_dst, project+attend on PE,
      exp(lrelu) on ACT, scatter msg||exp to a unique dst-sorted slot.
  P2: accumulation — read slots in dst order, dedup each 128-slot tile
      with a self-match matrix matmul, scatter one row per dst segment.
  P3: normalize (softmax denom), relu, semantic-attention partials and
      2-dim output projections.
Host: bucket/sort edges, assemble int16 index arrays; final 2-way
metapath softmax combine.
"""

import dataclasses
import time

import numpy as np
import ml_dtypes

import jax as _jax
try:
    _jax.config.update("jax_compilation_cache_dir", "/tmp/jax_cache_han")
    _jax.config.update("jax_persistent_cache_min_entry_size_bytes", -1)
    _jax.config.update("jax_persistent_cache_min_compile_time_secs", 0.0)
except Exception:
    pass

import concourse.bass as bass
import concourse.bacc as bacc
import concourse.mybir as mybir
import concourse.tile as tile
import bass_rust
from concourse.masks import make_identity

bf16 = np.float16
F32 = mybir.dt.float32
BF = mybir.dt.float16
I16 = mybir.dt.int16
AXX = bass_rust.AxisListType.X
N_CORES = 8
H, DH, F = 8, 16, 128
DIN = 64
MP = ("cs", "ss")


@dataclasses.dataclass
class P:
    NNODE: int = 100000          # nodes per type
    PART: int = 12500            # dst nodes per core
    PAIR_BANK: int = 16384       # gather idx per src bank (pairs)
    W: int = 16                  # dst nodes per 128-slot window
    WPB: int = 224               # windows per slot bank
    SLOT_BANK: int = 32768       # slot-idx space per scatter bank
    ACC_CHUNK: int = 4096        # slots per accumulation chunk
    CAPS_TBL: tuple = ((3328, 3328, 3328, 256), (3328, 3328, 3328, 256),
                       (3328, 3328, 3328, 256), (1792, 1792, 1792, 256))

    @property
    def N_SRC_BANK(self):
        return max(1, -(-(self.NNODE // 2) // self.PAIR_BANK))

    @property
    def N_WIN(self):
        return -(-self.PART // self.W)

    @property
    def N_SLOT_BANK(self):
        return -(-self.N_WIN // self.WPB)

    @property
    def SLOT_REAL(self):
        return self.WPB * 128

    def wb(self, b):
        return min(self.WPB, self.N_WIN - self.WPB * b)

    def chunks_b(self, b):
        return -(-self.wb(b) * 128 // self.ACC_CHUNK)

    @property
    def ACC_REGIONS(self):
        return [(b, ch) for b in range(self.N_SLOT_BANK)
                for ch in range(self.chunks_b(b))]

    @property
    def ASLOTS(self):
        return len(self.ACC_REGIONS) * self.ACC_CHUNK

    @property
    def N_GROUP(self):
        return self.N_SLOT_BANK * self.N_SRC_BANK * 2

    def capg(self, g):
        return self.CAPS_TBL[g // (self.N_SRC_BANK * 2)][
            (g // 2) % self.N_SRC_BANK]

    @property
    def CUMCAPS(self):
        import itertools
        return [0] + list(itertools.accumulate(
            self.capg(g) for g in range(self.N_GROUP)))

    @property
    def ECAP(self):
        return self.CUMCAPS[-1]

    @property
    def ACC_SUB(self):
        return self.ACC_CHUNK // 128

    @property
    def DST_TILES(self):
        return (self.PART + 127) // 128

    @property
    def LAST_TILE_N(self):
        return self.PART - (self.DST_TILES - 1) * 128

    @property
    def FT_ROWS(self):
        return self.DST_TILES * 128 + 128

    @property
    def DUMP_DST(self):
        return self.DST_TILES * 128 + 9

    @property
    def AD_PAD(self):
        return self.PART + 8


FULL = P()
SMALL = P(NNODE=2048, PART=256, PAIR_BANK=1024, W=16, WPB=6,
          SLOT_BANK=1024, ACC_CHUNK=1024,
          CAPS_TBL=((384,), (384,), (256,)))


def build(p: P, phases: int = 3, p1mode: int = 0):
    nc = bacc.Bacc("TRN2", target_bir_lowering=False, debug=False,
                   num_devices=N_CORES)
    NPAIR = p.NNODE // 2

    xs_sh = nc.dram_tensor("xs_sh", [p.PART, DIN], BF, kind="ExternalInput")
    xc_sh = nc.dram_tensor("xc_sh", [p.PART, DIN], BF, kind="ExternalInput")
    E16 = p.ECAP // 16
    ins = {}
    for m in MP:
        ins[f"eidx_{m}"] = nc.dram_tensor(f"eidx_{m}", [16, 3 * E16],
                                          I16, kind="ExternalInput")
        ins[f"scatidx_{m}"] = nc.dram_tensor(
            f"scatidx_{m}", [16, p.ASLOTS // 16], I16, kind="ExternalInput")
        ins[f"dstrow_{m}"] = nc.dram_tensor(
            f"dstrow_{m}", [p.ASLOTS // p.ACC_CHUNK, p.ACC_CHUNK], I16,
            kind="ExternalInput")
    cblob = nc.dram_tensor("cblob", [128, 691], BF, kind="ExternalInput")

    pout = nc.dram_tensor("pout", [2, 2 * p.DST_TILES * 128 + 128],
                          F32, kind="ExternalOutput")
    p_out = {m: pout[:, mi * p.DST_TILES * 128:
                     (mi + 1) * p.DST_TILES * 128]
             for mi, m in enumerate(MP)}

    xs_b = nc.dram_tensor("xs_b", [p.PART, DIN], BF)
    xc_b = nc.dram_tensor("xc_b", [p.PART, DIN], BF)
    xs_full = nc.dram_tensor("xs_full", [p.NNODE, DIN], BF,
                             addr_space="Shared")
    xc_full = nc.dram_tensor("xc_full", [p.NNODE, DIN], BF,
                             addr_space="Shared")
    ad_tab = nc.dram_tensor("ad_tab", [p.PART + 64, DIN], F32)
    inter = {m: nc.dram_tensor(f"inter_{m}", [p.N_SLOT_BANK * p.SLOT_BANK, 256], BF) for m in MP}
    ftab = {m: nc.dram_tensor(f"ftab_{m}", [p.FT_ROWS, 256], BF) for m in MP}
    rep = {}
    for m in MP:
        for nm, ncol in (("eidx", 3 * E16), ("scatidx", p.ASLOTS // 16)):
            rep[f"{nm}_{m}"] = nc.dram_tensor(f"r{nm}_{m}", [128, ncol], I16)

    x_full = {"cs": xc_full, "ss": xs_full}

    with tile.TileContext(nc) as tc:
        with (
            tc.tile_pool(name="consts", bufs=1) as cpool,
            tc.tile_pool(name="sbuf", bufs=2) as pool,
            tc.tile_pool(name="acc1", bufs=1) as apool,
            tc.tile_pool(name="psum", bufs=8, space="PSUM") as pp,
        ):
            # ================= P0 =================
            nc.sync.dma_start(out=xs_b[:], in_=xs_sh[:])
            nc.sync.dma_start(out=xc_b[:], in_=xc_sh[:])
            nc.gpsimd.collective_compute(
                "AllGather", mybir.AluOpType.bypass,
                replica_groups=[list(range(N_CORES))],
                ins=[xs_b[:]], outs=[xs_full[:]])
            nc.gpsimd.collective_compute(
                "AllGather", mybir.AluOpType.bypass,
                replica_groups=[list(range(N_CORES))],
                ins=[xc_b[:]], outs=[xc_full[:]])

            for m in MP:
                for nm in ("eidx", "scatidx"):
                    for k in range(8):
                        nc.sync.dma_start(
                            out=rep[f"{nm}_{m}"][16 * k:16 * (k + 1), :],
                            in_=ins[f"{nm}_{m}"][:])

            ident = cpool.tile([128, 128], BF)
            make_identity(nc, ident[:])
            identf = cpool.tile([128, 128], F32)
            make_identity(nc, identf[:])
            cb = cpool.tile([128, 691], BF)
            nc.sync.dma_start(out=cb[:], in_=cblob[:])
            wcat_t = {}
            for mi_, m in enumerate(MP):
                for par in range(2):
                    off = (mi_ * 2 + par) * 136
                    wcat_t[(m, par)] = cb[:, off:off + 136]
            wad_t = cb[0:DIN, 544:560]
            kw_t = cb[:, 560:688]
            linw_t = cb[:, 688:690]
            kb_t = cb[:, 690:691]

            # a_d table
            for t_i in range(p.DST_TILES):
                lo = t_i * 128
                n = min(128, p.PART - lo)
                xst = pool.tile([128, DIN], BF, tag="xst")
                if n < 128:
                    nc.vector.memset(xst[:], 0.0)
                nc.sync.dma_start(out=xst[:n, :], in_=xs_sh[lo:lo + n, :])
                tp = pp.tile([DIN, 128], BF, tag="ps")
                nc.tensor.transpose(out=tp[:], in_=xst[:], identity=ident[:])
                xstT = pool.tile([DIN, 128], BF, tag="xstT")
                nc.vector.tensor_copy(out=xstT[:], in_=tp[:])
                adp = pp.tile([128, 16], F32, tag="ps")
                nc.tensor.matmul(out=adp[:], lhsT=xstT[:], rhs=wad_t,
                                 start=True, stop=True)
                ads = pool.tile([128, 16], F32, tag="ads")
                nc.vector.tensor_copy(out=ads[:], in_=adp[:])
                nc.sync.dma_start(out=ad_tab[lo:lo + n, 0:16], in_=ads[:n, :])
            padt = cpool.tile([128, DIN], F32)
            nc.vector.memset(padt[:], -100.0)
            nc.sync.dma_start(out=ad_tab[p.PART:p.PART + 64, :],
                              in_=padt[0:64, :])

            zt = cpool.tile([128, 16, 256], BF)
            nc.vector.memset(zt[:], 0.0)
            for m in MP:
                iv = inter[m][:].rearrange("(a q) c -> q a c", q=128)
                for b in range(p.N_SLOT_BANK):
                    base = b * p.SLOT_BANK // 128
                    nreg = p.chunks_b(b) * p.ACC_CHUNK // 128
                    ndmp = (p.SLOT_BANK - p.SLOT_REAL) // 128
                    for k0, k1 in ((base, base + nreg),
                                   (base + p.SLOT_REAL // 128,
                                    base + p.SLOT_REAL // 128 + ndmp)):
                        for k in range(k0, k1, 16):
                            w = min(16, k1 - k)
                            nc.sync.dma_start(out=iv[:, k:k + w, :],
                                              in_=zt[:, :w, :])
                fv = ftab[m][:].rearrange("(a q) c -> q a c", q=128)
                nfa = p.FT_ROWS // 128
                for k in range(0, nfa, 16):
                    w = min(16, nfa - k)
                    nc.sync.dma_start(out=fv[:, k:k + w, :],
                                      in_=zt[:, :w, :])

            # ================= P1: edge phase =================
            for m in (MP if phases >= 1 else ()):
                NSB = p.N_SRC_BANK
                for g in range(p.N_GROUP):
                    sb_bank = g // (2 * NSB)
                    srcb = (g // 2) % NSB
                    par = g % 2
                    CAP = p.capg(g)
                    EPS = CAP // 128
                    c0 = p.CUMCAPS[g] // 16
                    ncol = CAP // 16
                    E16_ = p.ECAP // 16
                    gi = pool.tile([128, ncol], I16, tag="gi")
                    nc.sync.dma_start(
                        out=gi[:], in_=rep[f"eidx_{m}"][:, c0:c0 + ncol])
                    di = pool.tile([128, ncol], I16, tag="di")
                    nc.sync.dma_start(
                        out=di[:],
                        in_=rep[f"eidx_{m}"][:, E16_ + c0:E16_ + c0 + ncol])
                    si = pool.tile([128, ncol], I16, tag="si")
                    nc.sync.dma_start(
                        out=si[:],
                        in_=rep[f"eidx_{m}"][:, 2 * E16_ + c0:
                                             2 * E16_ + c0 + ncol])

                    xg = pool.tile([128, EPS, 128], BF, tag="xg")
                    bank_ap = bass.AP(
                        x_full[m][:].tensor, srcb * p.PAIR_BANK * 128,
                        [[128, NPAIR - srcb * p.PAIR_BANK], [1, 128]])
                    if p1mode < 2:
                        nc.gpsimd.dma_gather(
                            xg[:], bank_ap, gi[:], CAP, CAP, 128,
                            single_packet=False)
                    else:
                        nc.vector.memset(xg[:], 0.25)
                    ad = pool.tile([128, EPS, DIN], F32, tag="ad")
                    if p1mode < 1:
                        nc.gpsimd.dma_gather(
                            ad[:], ad_tab[:], di[:], CAP, CAP, DIN,
                            single_packet=False)
                    else:
                        nc.vector.memset(ad[:], 0.25)

                    talpha = pool.tile([128, EPS * 8], F32, tag="ta")
                    etile = pool.tile([128, EPS * 8], F32, tag="et")
                    scb = pool.tile([128, EPS, 256], BF, tag="scb")
                    adoff = 0 if m == "cs" else 8
                    GB = 4  # psum group batch
                    for s0 in range(0, EPS, GB):
                        s1 = min(s0 + GB, EPS)
                        hps = []
                        for s in range(s0, s1):
                            xtp = pp.tile([128, 128], BF, tag="ps")
                            nc.tensor.transpose(out=xtp[:], in_=xg[:, s, :],
                                                identity=ident[:])
                            xts = pool.tile([128, 128], BF, tag="xts")
                            nc.vector.tensor_copy(out=xts[:], in_=xtp[:])
                            hp = pp.tile([128, 136], F32, tag="ps")
                            nc.tensor.matmul(
                                out=hp[:], lhsT=xts[:],
                                rhs=wcat_t[(m, par)],
                                start=True, stop=True)
                            nc.vector.tensor_tensor(
                                out=talpha[:, 8 * s:8 * (s + 1)],
                                in0=hp[:, 128:136],
                                in1=ad[:, s, adoff:adoff + 8],
                                op=mybir.AluOpType.add)
                            hps.append(hp)
                        nsub = s1 - s0
                        tsl = talpha[:, 8 * s0:8 * s1]
                        usl = etile[:, 8 * s0:8 * s1]
                        nc.vector.tensor_scalar_mul(usl, tsl, 0.2)
                        nc.vector.tensor_tensor(out=tsl, in0=tsl, in1=usl,
                                                op=mybir.AluOpType.max)
                        nc.scalar.activation(
                            out=usl, in_=tsl,
                            func=mybir.ActivationFunctionType.Exp)
                        esl = scb[:, s0, 128:136]
                        eout = bass.AP(esl.tensor, esl.offset,
                                       [esl.ap[0], [256, nsub], [1, 8]])
                        nc.vector.tensor_copy(out=eout,
                                              in_=etile[:, 8 * s0:8 * s1])
                        for s in range(s0, s1):
                            ea = etile[:, 8 * s:8 * s + 8]
                            ebc = bass.AP(ea.tensor, ea.offset,
                                          [ea.ap[0], [1, 8], [0, 16]])
                            nc.vector.tensor_tensor(
                                out=scb[:, s, 0:128],
                                in0=hps[s - s0][:, 0:128], in1=ebc,
                                op=mybir.AluOpType.mult)
                    if p1mode < 3:
                        nc.gpsimd.dma_scatter_add(
                            inter[m][sb_bank * p.SLOT_BANK:
                                     (sb_bank + 1) * p.SLOT_BANK, :],
                            scb[:], si[:], CAP, CAP, 256,
                            single_packet=False)

            # ================= P2: accumulation =================
            for m in (MP if phases >= 2 else ()):
                dcol_i = apool.tile([128, p.ASLOTS // 128], I16,
                                    tag=f"dcoli{m}")
                nc.sync.dma_start(
                    out=dcol_i[:],
                    in_=ins[f"dstrow_{m}"][:].rearrange(
                        "a (b c) -> (a b) c", c=128),
                    transpose=True)
                for ch, (rb, rch) in enumerate(p.ACC_REGIONS):
                    a0 = (rb * p.SLOT_BANK + rch * p.ACC_CHUNK) // 128
                    me = pool.tile([128, p.ACC_SUB, 256], BF, tag="me")
                    nc.sync.dma_start(
                        out=me[:],
                        in_=inter[m][:].rearrange("(a q) c -> q a c", q=128)[
                            :, a0:a0 + p.ACC_SUB, :])
                    sci = pool.tile([128, p.ACC_CHUNK // 16], I16, tag="sci")
                    nc.sync.dma_start(
                        out=sci[:],
                        in_=rep[f"scatidx_{m}"][
                            :, ch * (p.ACC_CHUNK // 16):
                            (ch + 1) * (p.ACC_CHUNK // 16)])
                    rt = pool.tile([128, p.ACC_CHUNK], I16, tag="rt")
                    rsrc = bass.AP(ins[f"dstrow_{m}"][:].tensor,
                                   ch * p.ACC_CHUNK,
                                   [[0, 128], [1, p.ACC_CHUNK]])
                    nc.sync.dma_start(out=rt[:], in_=rsrc)
                    scat = pool.tile([128, p.ACC_SUB, 256], BF, tag="scat")
                    for s in range(p.ACC_SUB):
                        st = ch * p.ACC_SUB + s
                        csl = dcol_i[:, st:st + 1]
                        col_bc = bass.AP(csl.tensor, csl.offset,
                                         [csl.ap[0], [0, 128]])
                        S = pool.tile([128, 128], BF, tag="S")
                        nc.vector.tensor_tensor(
                            out=S[:], in0=col_bc,
                            in1=rt[:, 128 * s:128 * (s + 1)],
                            op=mybir.AluOpType.is_equal)
                        dp = pp.tile([128, 136], F32, tag="ps")
                        nc.tensor.matmul(out=dp[:], lhsT=S[:],
                                         rhs=me[:, s, 0:136],
                                         start=True, stop=True)
                        nc.vector.tensor_copy(out=scat[:, s, 0:136],
                                              in_=dp[:])
                    nc.gpsimd.dma_scatter_add(
                        ftab[m][:], scat[:], sci[:],
                        p.ACC_CHUNK, p.ACC_CHUNK, 256,
                        single_packet=False)

            # ================= P3: epilogue =================
            tsum_t = apool.tile([128, 2], F32, tag="tsum")
            nc.vector.memset(tsum_t[:], 0.0)
            for mi, m in enumerate(MP if phases >= 3 else ()):
                for t_i in range(p.DST_TILES):
                    ft = pool.tile([128, 136], BF, tag="ft")
                    nc.sync.dma_start(
                        out=ft[:],
                        in_=ftab[m][:].rearrange("(a q) c -> q a c", q=128)[
                            :, t_i, 0:136])
                    sden = pool.tile([128, 8], F32, tag="sden")
                    nc.vector.tensor_scalar_add(sden[:], ft[:, 128:136],
                                                1e-16)
                    rec = pool.tile([128, 8], F32, tag="rec")
                    nc.vector.reciprocal(rec[:], sden[:])
                    o32 = pool.tile([128, 128], F32, tag="o32")
                    rbc = bass.AP(rec.tensor, rec[:].offset,
                                  [rec[:].ap[0], [1, 8], [0, 16]])
                    nc.vector.tensor_tensor(out=o32[:], in0=ft[:, 0:128],
                                            in1=rbc,
                                            op=mybir.AluOpType.mult)
                    ob = pool.tile([128, 128], BF, tag="ob")
                    nc.vector.tensor_scalar_max(ob[:], o32[:], 0.0)
                    otp = pp.tile([128, 128], BF, tag="ps")
                    nc.tensor.transpose(out=otp[:], in_=ob[:],
                                        identity=ident[:])
                    otb = pool.tile([128, 128], BF, tag="otb")
                    nc.vector.tensor_copy(out=otb[:], in_=otp[:])
                    ktp = pp.tile([128, 128], F32, tag="ps")
                    nc.tensor.matmul(out=ktp[:], lhsT=kw_t, rhs=otb[:],
                                     start=True, stop=True)
                    th = pool.tile([128, 128], F32, tag="th")
                    nc.scalar.activation(
                        out=th[:], in_=ktp[:],
                        func=mybir.ActivationFunctionType.Tanh,
                        bias=kb_t)
                    red = pool.tile([128, 1], F32, tag="red")
                    nlast = p.LAST_TILE_N if t_i == p.DST_TILES - 1 else 128
                    nc.vector.reduce_sum(red[:], th[:, 0:nlast], axis=AXX)
                    nc.vector.tensor_add(out=tsum_t[:, mi:mi + 1],
                                         in0=tsum_t[:, mi:mi + 1],
                                         in1=red[:])
                    ppm = pp.tile([2, 128], F32, tag="ps")
                    nc.tensor.matmul(out=ppm[:], lhsT=linw_t, rhs=otb[:],
                                     start=True, stop=True)
                    po = pool.tile([2, 128], F32, tag="po")
                    nc.vector.tensor_copy(out=po[:], in_=ppm[:])
                    nc.sync.dma_start(
                        out=p_out[m][:, 128 * t_i:128 * (t_i + 1)],
                        in_=po[:])
            tspm = pp.tile([2, 128], F32, tag="ps")
            nc.tensor.matmul(out=tspm[:], lhsT=tsum_t[:], rhs=identf[:],
                             start=True, stop=True)
            tss = pool.tile([2, 128], F32, tag="tss")
            nc.vector.tensor_copy(out=tss[:], in_=tspm[:])
            nc.sync.dma_start(
                out=pout[:, 2 * p.DST_TILES * 128:2 * p.DST_TILES * 128 + 128],
                in_=tss[:])

    nc.finalize()
    return nc


# ====================== host prep ======================

_TMPL = {}


def _pad_templates(p):
    if "sidx" not in _TMPL:
        NE = N_CORES * p.ECAP
        _TMPL["sidx"] = (
            p.SLOT_REAL +
            (np.arange(NE) % (p.SLOT_BANK - p.SLOT_REAL))).astype(np.int16)
        _TMPL["didx"] = np.full(NE, p.AD_PAD, np.int16)
        _TMPL["dstslot"] = np.full(N_CORES * p.ASLOTS, -1, np.int16)
        _TMPL["scat"] = np.full(N_CORES * p.ASLOTS, p.DUMP_DST, np.int16)
    return _TMPL


def prep_metapath(src, dst, p: P):
    E = src.shape[0]
    src = np.asarray(src, np.int32)
    dst = np.asarray(dst, np.int32)
    o_lo = np.argsort(dst.astype(np.uint16), kind="stable")
    hi_s = (dst >> 16).astype(np.uint8)[o_lo]
    o1 = np.concatenate([o_lo[hi_s == 0], o_lo[hi_s == 1]])
    d1 = dst[o1]
    s1 = src[o1]
    c1 = d1 // p.PART
    dloc = d1 - c1 * p.PART

    newseg = np.empty(E, bool)
    newseg[0] = True
    newseg[1:] = d1[1:] != d1[:-1]

    st = dloc // p.W                      # window within core
    win_g = c1 * p.N_WIN + st
    counts = np.bincount(win_g, minlength=N_CORES * p.N_WIN)
    assert counts.max() <= 128, counts.max()
    wstart = np.concatenate(
        [[0], np.cumsum(counts)])[:-1].astype(np.int32)
    r = np.arange(E, dtype=np.int32) - wstart[win_g]

    bank = st // p.WPB
    stb = st - bank * p.WPB
    sidx16 = stb * 128 + r                # < SLOT_REAL
    abase = np.cumsum([0] + [p.chunks_b(b) * p.ACC_CHUNK
                             for b in range(p.N_SLOT_BANK)])
    aslot = abase[bank] + stb * 128 + r   # position in accumulation space

    NSB = p.N_SRC_BANK
    srcbank = s1 // (2 * p.PAIR_BANK)
    par = s1 & 1
    grp = (bank * NSB + srcbank) * 2 + par
    key2 = (c1 * p.N_GROUP + grp).astype(np.int32)
    assert p.N_GROUP * N_CORES <= 256
    o2 = np.argsort(key2.astype(np.uint8), kind="stable")
    g2 = key2[o2]
    gcounts = np.bincount(g2, minlength=N_CORES * p.N_GROUP)
    capg_all = np.array([p.capg(g % p.N_GROUP)
                         for g in range(N_CORES * p.N_GROUP)])

    assert (gcounts <= capg_all).all(), \
        (gcounts - capg_all).max()
    cum_all = np.concatenate(
        [[0], np.cumsum(np.tile([p.capg(g) for g in range(p.N_GROUP)],
                                N_CORES))])
    gstart = np.concatenate(
        [[0], np.cumsum(gcounts)]).astype(np.int32)
    rank = np.arange(E, dtype=np.int32) - gstart[g2]
    eidx = cum_all[g2].astype(np.int64) + rank

    NE = N_CORES * p.ECAP
    tmpl = _pad_templates(p)
    gidx = np.zeros(NE, np.int16)
    didx = tmpl["didx"].copy()
    sidx = tmpl["sidx"].copy()
    gidx[eidx] = ((s1[o2] >> 1) - srcbank[o2] * p.PAIR_BANK).astype(np.int16)
    didx[eidx] = dloc[o2].astype(np.int16)
    sidx[eidx] = sidx16[o2].astype(np.int16)

    def wrap16(a):  # [NCORES, N] -> [NCORES, 16, N//16]
        return np.ascontiguousarray(
            a.reshape(N_CORES, -1, 16).transpose(0, 2, 1))

    eidx = np.concatenate([wrap16(gidx.reshape(N_CORES, -1)),
                           wrap16(didx.reshape(N_CORES, -1)),
                           wrap16(sidx.reshape(N_CORES, -1))], axis=2)

    gaslot = c1.astype(np.int64) * p.ASLOTS + aslot
    dstslot = tmpl["dstslot"].copy()
    dstslot[gaslot] = dloc.astype(np.int16)
    scat = tmpl["scat"].copy()
    scat[gaslot[newseg]] = dloc[newseg].astype(np.int16)
    scatw = wrap16(scat.reshape(N_CORES, -1))
    drow = dstslot.reshape(N_CORES, p.ASLOTS // p.ACC_CHUNK, p.ACC_CHUNK)
    return dict(eidx=eidx, scatidx=scatw, dstrow=drow)


def prep_all(inputs, p: P, skip_x=False):
    e_cs = np.asarray(inputs["edge_cs"])
    e_ss = np.asarray(inputs["edge_ss"])
    W_s = np.asarray(inputs["W_subj"], np.float32)
    W_c = np.asarray(inputs["W_chan"], np.float32)

    mp_prep = {
        "cs": prep_metapath(e_cs[0], e_cs[1], p),
        "ss": prep_metapath(e_ss[0], e_ss[1], p),
    }
    if not skip_x:
        xs_bf = np.asarray(inputs["x_subject"], np.float32).astype(bf16)
        xc_bf = np.asarray(inputs["x_channel"], np.float32).astype(bf16)

    wcat = {}
    for m, W, att_s in (("cs", W_c, inputs["att_src_cs"]),
                        ("ss", W_s, inputs["att_src_ss"])):
        wa = np.einsum("khd,hd->kh", W.reshape(DIN, H, DH),
                       np.asarray(att_s, np.float32))
        cat = np.concatenate([W, wa], axis=1)
        for parn in range(2):
            z = np.zeros((128, 136), np.float32)
            z[64 * parn:64 * (parn + 1), :] = cat
            wcat[(m, parn)] = z.astype(bf16)
    wad = np.concatenate(
        [np.einsum("khd,hd->kh", W_s.reshape(DIN, H, DH),
                   np.asarray(inputs["att_dst_cs"], np.float32)),
         np.einsum("khd,hd->kh", W_s.reshape(DIN, H, DH),
                   np.asarray(inputs["att_dst_ss"], np.float32))],
        axis=1).astype(bf16)
    kw = np.asarray(inputs["k_w"], np.float32).astype(bf16)
    kb = np.ascontiguousarray(
        np.asarray(inputs["k_b"], np.float32).reshape(128, 1))
    linw = np.asarray(inputs["lin_w"], np.float32).astype(bf16)

    in_maps = []
    for c in range(N_CORES):
        im = {
            "wad": wad, "kw": kw, "kb": kb, "linw": linw,
        }
        if not skip_x:
            im["xs_sh"] = np.ascontiguousarray(
                xs_bf[c * p.PART:(c + 1) * p.PART])
            im["xc_sh"] = np.ascontiguousarray(
                xc_bf[c * p.PART:(c + 1) * p.PART])
        for m in MP:
            pr = mp_prep[m]
            for nm in ("gidx", "didx", "sidx", "scatidx", "dstrow"):
                im[f"{nm}_{m}"] = pr[nm][c]
            for parn in range(2):
                im[f"wcat_{m}{parn}"] = wcat[(m, parn)]
        in_maps.append(im)
    return in_maps


def epilogue(results, inputs, p: P):
    NS = p.NNODE
    q = np.asarray(inputs["q"], np.float32)
    lin_b = np.asarray(inputs["lin_b"], np.float32)
    D2 = 2 * p.DST_TILES * 128
    tsum = sum(np.asarray(r["pout"], np.float32)[:, D2:D2 + 128].T
               for r in results)
    score = (tsum / np.float32(NS)).T @ q
    score = score - score.max()
    attn = np.exp(score)
    attn /= attn.sum()
    D = p.DST_TILES * 128
    out = np.empty((NS, 2), np.float32)
    for c, r in enumerate(results):
        po = np.asarray(r["pout"], np.float32)
        pc = po[:, :p.PART]
        ps = po[:, D:D + p.PART]
        out[c * p.PART:(c + 1) * p.PART] = (attn[0] * pc + attn[1] * ps).T
    return out + lin_b


# ====================== persistent runner ======================

_RT = None
_PREZ = None
from concurrent.futures import ThreadPoolExecutor as _TPE
_XFER = _TPE(1)


def _make_runner(nc):
    import jax
    import jax.core
    from jax.sharding import Mesh, PartitionSpec
    from jax.experimental.shard_map import shard_map
    from concourse.bass2jax import (
        _bass_exec_p, install_neuronx_cc_hook, partition_id_tensor)

    install_neuronx_cc_hook()
    part_name = (nc.partition_id_tensor.name
                 if nc.partition_id_tensor else None)
    in_names, out_names, out_avals = [], [], []
    for alloc in nc.m.functions[0].allocations:
        if not isinstance(alloc, mybir.MemoryLocationSet):
            continue
        name = alloc.memorylocations[0].name
        if alloc.kind == "ExternalInput":
            if name != part_name:
                in_names.append(name)
        elif alloc.kind == "ExternalOutput":
            out_names.append(name)
            out_avals.append(jax.core.ShapedArray(
                tuple(alloc.tensor_shape), mybir.dt.np(alloc.dtype)))
    n_params = len(in_names)
    donate = tuple(range(n_params, n_params + len(out_names)))

    bind_names = in_names + out_names + ([part_name] if part_name else [])

    def _body(*args):
        ops = list(args)
        if part_name is not None:
            ops.append(partition_id_tensor())
        return tuple(_bass_exec_p.bind(
            *ops, out_avals=tuple(out_avals),
            in_names=tuple(bind_names),
            out_names=tuple(out_names),
            lowering_input_output_aliases=(),
            sim_require_finite=False, sim_require_nnan=False, nc=nc))

    devices = jax.devices()[:N_CORES]
    assert len(devices) >= N_CORES
    mesh = Mesh(np.asarray(devices[:N_CORES]), ("core",))
    nio = n_params + len(out_names)
    sharded = jax.jit(
        shard_map(_body, mesh=mesh,
                  in_specs=(PartitionSpec("core"),) * nio,
                  out_specs=(PartitionSpec("core"),) * len(out_names),
                  check_rep=False),
        donate_argnums=donate, keep_unused=True)
    from jax.sharding import NamedSharding
    shd = NamedSharding(mesh, PartitionSpec("core"))

    import jax.numpy as jnp

    @jax.jit
    def _mkzeros():
        return tuple(
            jnp.zeros((N_CORES * a.shape[0], *a.shape[1:]), a.dtype)
            for a in out_avals)

    mkzeros = jax.jit(_mkzeros, out_shardings=(shd,) * len(out_avals))
    return sharded, in_names, out_names, out_avals, shd, mkzeros


def _ensure_runtime():
    global _RT
    if _RT is None:
        nc = build(FULL)
        _RT = _make_runner(nc)
    return _RT


def _run_device(in_maps):
    sharded, in_names, out_names, out_avals, _, _mz = _ensure_runtime()
    concat_in = [np.concatenate([in_maps[c][nm] for c in range(N_CORES)],
                                axis=0) for nm in in_names]
    concat_zeros = [np.zeros((N_CORES * a.shape[0], *a.shape[1:]), a.dtype)
                    for a in out_avals]
    outs = sharded(*concat_in, *concat_zeros)
    results = []
    for c in range(N_CORES):
        results.append({nm: np.asarray(o).reshape(
            N_CORES, *out_avals[i].shape)[c]
            for i, (nm, o) in enumerate(zip(out_names, outs))})
    return results


def _input_shapes(p):
    sh = {
        "xs_sh": ((p.PART, DIN), np.float16),
        "xc_sh": ((p.PART, DIN), np.float16),
        "wad": ((DIN, 16), np.float16),
        "kw": ((128, 128), np.float16),
        "kb": ((128, 1), np.float32),
        "linw": ((128, 2), np.float16),
    }
    for m in MP:
        sh[f"gidx_{m}"] = ((16, p.ECAP // 16), np.int16)
        sh[f"didx_{m}"] = ((16, p.ECAP // 16), np.int16)
        sh[f"sidx_{m}"] = ((16, p.ECAP // 16), np.int16)
        sh[f"scatidx_{m}"] = ((16, p.ASLOTS // 16), np.int16)
        sh[f"dstrow_{m}"] = ((p.ASLOTS // p.ACC_CHUNK, p.ACC_CHUNK), np.int16)
        for parn in range(2):
            sh[f"wcat_{m}{parn}"] = ((128, 136), np.float16)
    return sh


def _warmup():
    """Compile + NEFF load + one dummy end-to-end call at import time."""
    p = FULL
    ar = np.arange(500000, dtype=np.int64)
    ed = (ar % p.NNODE).astype(np.int32)[None, :]
    es = ((ar * 40503 + 12345) % p.NNODE).astype(np.int32)[None, :]
    dummy = {
        "x_subject": np.zeros((p.NNODE, DIN), np.float32),
        "x_channel": np.zeros((p.NNODE, DIN), np.float32),
        "edge_cs": np.concatenate([es, ed]), "edge_ss": np.concatenate([es, ed]),
        "W_subj": np.zeros((DIN, 128), np.float32),
        "b_subj": np.zeros(128, np.float32),
        "W_chan": np.zeros((DIN, 128), np.float32),
        "b_chan": np.zeros(128, np.float32),
        "att_src_cs": np.zeros((H, DH), np.float32),
        "att_dst_cs": np.zeros((H, DH), np.float32),
        "att_src_ss": np.zeros((H, DH), np.float32),
        "att_dst_ss": np.zeros((H, DH), np.float32),
        "k_w": np.zeros((128, 128), np.float32),
        "k_b": np.zeros(128, np.float32),
        "q": np.zeros(128, np.float32),
        "lin_w": np.zeros((128, 2), np.float32),
        "lin_b": np.zeros(2, np.float32),
    }
    kernel(**dummy)


def _consts_blob(inputs):
    """[128, 691] f16: 4x wcat(136) | wad(16, rows 0:64) | kw(128) |
    linw(2) | kb(1)."""
    W_s = np.asarray(inputs["W_subj"], np.float32)
    W_c = np.asarray(inputs["W_chan"], np.float32)
    blob = np.zeros((128, 691), np.float32)
    for mi, (Wm, att_s) in enumerate(((W_c, inputs["att_src_cs"]),
                                      (W_s, inputs["att_src_ss"]))):
        wa = np.einsum("khd,hd->kh", Wm.reshape(DIN, H, DH),
                       np.asarray(att_s, np.float32))
        cat = np.concatenate([Wm, wa], axis=1)
        for parn in range(2):
            off = (mi * 2 + parn) * 136
            blob[64 * parn:64 * (parn + 1), off:off + 136] = cat
    blob[0:DIN, 544:560] = np.concatenate(
        [np.einsum("khd,hd->kh", W_s.reshape(DIN, H, DH),
                   np.asarray(inputs["att_dst_cs"], np.float32)),
         np.einsum("khd,hd->kh", W_s.reshape(DIN, H, DH),
                   np.asarray(inputs["att_dst_ss"], np.float32))],
        axis=1)
    blob[:, 560:688] = np.asarray(inputs["k_w"], np.float32)
    blob[:, 688:690] = np.asarray(inputs["lin_w"], np.float32)
    blob[:, 690] = np.asarray(inputs["k_b"], np.float32)
    return blob.astype(np.float16)


def kernel(**inputs):
    import jax
    p = FULL
    sharded, in_names, out_names, out_avals, shd, mkzeros = _ensure_runtime()
    # start the big x transfers first (async), overlap with host prep
    xs16 = np.asarray(np.asarray(inputs["x_subject"], np.float32)
                      .astype(np.float16))
    xc16 = np.asarray(np.asarray(inputs["x_channel"], np.float32)
                      .astype(np.float16))
    dev = {"xs_sh": _XFER.submit(jax.device_put, xs16, shd),
           "xc_sh": _XFER.submit(jax.device_put, xc16, shd)}
    # per-metapath prep; upload each metapath's arrays as soon as ready so
    # the second metapath's host prep overlaps the first one's transfer
    edges = {"cs": np.asarray(inputs["edge_cs"]),
             "ss": np.asarray(inputs["edge_ss"])}
    for m in MP:
        pr = prep_metapath(edges[m][0], edges[m][1], p)
        for nm in ("eidx", "scatidx", "dstrow"):
            a = pr[nm]
            dev[f"{nm}_{m}"] = _XFER.submit(
                jax.device_put,
                np.ascontiguousarray(a.reshape(-1, a.shape[-1])), shd)
    consts = {"cblob": _consts_blob(inputs)}
    args = []
    for nm in in_names:
        if nm in dev:
            args.append(dev[nm].result())
        else:
            args.append(np.concatenate([consts[nm]] * N_CORES, axis=0))
    global _PREZ
    zeros = _PREZ if _PREZ is not None else mkzeros()
    _PREZ = None
    outs = sharded(*args, *zeros)
    _PREZ = mkzeros()   # async refill for the next call
    fetched = jax.device_get(list(outs))
    results = []
    for c in range(N_CORES):
        results.append({nm: fetched[i].reshape(
            N_CORES, *out_avals[i].shape)[c]
            for i, nm in enumerate(out_names)})
    return epilogue(results, inputs, p)


_warmup()
